# revision 11
# baseline (speedup 1.0000x reference)
"""Trainium2 Bass kernel for ConfidenceGCNConv message passing.

Math (reference):
    w_e   = sigmoid(edge_attr @ conf_w.T + conf_b)            # [E]
    deg   = bincount(row); dis = where(deg>0, rsqrt(deg), 0)  # [N]
    out[c] = sum_{e: col_e=c} dis[row_e]*dis[col_e]*w_e * x[row_e]
    y     = out @ lin_w.T + lin_b

Device strategy (8 cores, destination-partitioned):
  - Host: partition edges by destination block (128 dests / block, 49 blocks
    per core), split per block into row<HALF / row>=HALF halves (int16 gather
    index limit), pad each half to whole 128-lane chunks. Pure index work.
  - Device per core, per destination block b:
      * dma_gather x16[row_e] rows (fp16, 1KB each) -> Xg [128e, C, 512]
      * dma_gather dis_wide[row_e] (256B rows, col 0 = dis[row]) -> Dr
      * Wsel[e, d] = w_e * dis[row_e] * (col_local_e == d)   (DVE iota-compare)
      * PSUM spmm[128d, 512k] += sum_c Wsel_c^T @ Xg_c       (PE, contraction=edges)
      * transpose spmm -> outT [128k, 4, 128d]               (PE identity trick)
      * PSUM y[128d, 512o] = sum_kc outT_kc^T @ WT_kc        (PE)
      * y_sb = y * dis[dest] + bias                          (scale commutes with @W)
  - w_e = sigmoid(...) and dis = masked 1/sqrt(deg) computed on device.
    dis_wide (replicated 128-wide dis table for 256B-aligned gathers) is
    built on device once per core.
"""

import sys

for _p in ("/opt/trn_rl_repo",):
    if _p not in sys.path:
        sys.path.insert(0, _p)

import numpy as np

import concourse.bass as bass
import concourse.mybir as mybir
import concourse.tile as tile
from concourse import bacc
from concourse.bass_utils import run_bass_kernel_spmd

P = 128
NCORES = 8
F16 = mybir.dt.float16
F32 = mybir.dt.float32
I16 = mybir.dt.int16


def _cdiv(a, b):
    return (a + b - 1) // b


def make_plan(edge_index, n_nodes, n_cores=NCORES):
    """Host-side integer/index preprocessing: edge partition + padded layout.

    Returns a dict with static program metadata (shared across cores) and the
    per-edge placement (core, lane) used to build per-core input arrays.
    """
    row = np.asarray(edge_index[0], dtype=np.int64)
    col = np.asarray(edge_index[1], dtype=np.int64)
    E = row.shape[0]

    NB = _cdiv(n_nodes, P)           # blocks covering all nodes
    BPC = _cdiv(NB, n_cores)         # blocks per core
    NBP = BPC * n_cores              # padded total blocks
    NPAD = NBP * P                   # padded node count (dest side)
    HALF = n_nodes // 2
    assert HALF <= 32768 and (n_nodes - HALF) <= 32768

    gb = col // P                    # global destination block
    core = gb // BPC
    b = gb - core * BPC              # block local to core
    half = (row >= HALF).astype(np.int64)
    key = (core * BPC + b) * 2 + half

    order = np.argsort(key, kind="stable")
    counts = np.bincount(key, minlength=2 * NBP)
    cnt3 = counts.reshape(n_cores, BPC, 2)
    chunks_needed = _cdiv(cnt3, P)
    C_LO = chunks_needed[:, :, 0].max(axis=0)  # [BPC], shared across cores
    C_HI = chunks_needed[:, :, 1].max(axis=0)
    C = C_LO + C_HI
    choff = np.zeros(BPC + 1, dtype=np.int64)
    np.cumsum(C, out=choff[1:])
    CH_TOT = int(choff[-1])
    L_TOT = CH_TOT * P

    # lane base per key (core-independent): lo at choff[b]*128, hi after lo
    lane_lo = choff[:BPC] * P
    lane_hi = lane_lo + C_LO * P
    lane_base_k = np.zeros(2 * NBP, dtype=np.int64)
    for c in range(n_cores):
        lane_base_k[(c * BPC + np.arange(BPC)) * 2 + 0] = lane_lo
        lane_base_k[(c * BPC + np.arange(BPC)) * 2 + 1] = lane_hi

    sorted_key = key[order]
    starts = np.zeros(2 * NBP, dtype=np.int64)
    np.cumsum(counts[:-1], out=starts[1:])
    pos_in_grp = np.arange(E, dtype=np.int64) - starts[sorted_key]
    lane = lane_base_k[sorted_key] + pos_in_grp
    edge_core = sorted_key // (2 * BPC)

    return dict(
        n_nodes=n_nodes, E=E, NB=NB, BPC=BPC, NBP=NBP, NPAD=NPAD, HALF=HALF,
        C_LO=C_LO.astype(int), C_HI=C_HI.astype(int), C=C.astype(int),
        choff=choff.astype(int), CH_TOT=CH_TOT, L_TOT=L_TOT,
        order=order, lane=lane, edge_core=edge_core,
        n_cores=n_cores,
    )


def make_core_arrays(plan, edge_index, edge_attr):
    """Per-core padded gather-index / col-local / edge-attr arrays."""
    row = np.asarray(edge_index[0], dtype=np.int64)
    col = np.asarray(edge_index[1], dtype=np.int64)
    HALF = plan["HALF"]
    CH_TOT = plan["CH_TOT"]
    L_TOT = plan["L_TOT"]
    out = []
    for c in range(plan["n_cores"]):
        sel = plan["edge_core"] == c
        e_sel = plan["order"][sel]
        lanes = plan["lane"][sel]
        rows = row[e_sel]
        idxval = np.where(rows < HALF, rows, rows - HALF).astype(np.int16)

        idx2d = np.zeros((16, L_TOT // 16), dtype=np.int16)
        idx2d[lanes % 16, lanes // 16] = idxval
        # HW reads the 16-partition wrap replicated across all 8 Q7 core
        # groups (sim only reads partitions 0-15; HW cores read their own).
        idx2d = np.tile(idx2d, (8, 1))

        cloc = np.full((P, CH_TOT), float(P), dtype=np.float16)  # 128 = sentinel
        cloc[lanes % P, lanes // P] = (col[e_sel] % P).astype(np.float16)

        attr3 = np.zeros((3, P, CH_TOT), dtype=np.float32)
        attr3[:, lanes % P, lanes // P] = np.asarray(edge_attr, np.float32)[e_sel].T

        out.append(dict(idx=idx2d, cloc=cloc, attr=attr3))
    return out


def build_program(plan, d_in, d_out, debug_mode=()):
    """Build the (SPMD-shared) Bass program.

    debug_mode: set of strings to disable pieces for HW bisection:
      'no_gather_x'  - memset xg instead of dma_gather from x16
      'no_gather_d'  - memset dr instead of dma_gather from dis_wide
      'no_diswide'   - skip dis_wide build
      'no_pe'        - skip matmul/transpose; y = bias only
    """
    debug_mode = set(debug_mode)
    n_nodes = plan["n_nodes"]
    NPAD, NBP, BPC = plan["NPAD"], plan["NBP"], plan["BPC"]
    HALF = plan["HALF"]
    C_LO, C_HI, C, choff = plan["C_LO"], plan["C_HI"], plan["C"], plan["choff"]
    CH_TOT, L_TOT = plan["CH_TOT"], plan["L_TOT"]
    CMAX = int(max(C))
    KC = d_in // P          # feature chunks (4)
    assert d_in % P == 0 and d_out % P == 0

    nc = bacc.Bacc("TRN2")

    x16 = nc.declare_dram_parameter("x16", [n_nodes, d_in], F16, isOutput=False)
    degt = nc.declare_dram_parameter("degt", [P, NBP], F32, isOutput=False)
    dego = nc.declare_dram_parameter("dego", [P, BPC], F32, isOutput=False)
    idxp = nc.declare_dram_parameter("idx", [P, L_TOT // 16], I16, isOutput=False)
    clocp = nc.declare_dram_parameter("cloc", [P, CH_TOT], F16, isOutput=False)
    attrp = nc.declare_dram_parameter("attr", [3, P, CH_TOT], F32, isOutput=False)
    confp = nc.declare_dram_parameter("confr", [P, 4], F32, isOutput=False)
    wtp = nc.declare_dram_parameter("wt", [d_in, d_out], F16, isOutput=False)
    biasp = nc.declare_dram_parameter("biasr", [P, d_out], F32, isOutput=False)
    iotap = nc.declare_dram_parameter("iota", [P, P], F16, isOutput=False)
    identp = nc.declare_dram_parameter("ident", [P, P], F16, isOutput=False)
    yout = nc.declare_dram_parameter("y", [BPC * P, d_out], F32, isOutput=True)

    dis_wide = nc.dram_tensor("dis_wide", [NPAD, P], F16)

    with tile.TileContext(nc) as tc:
        with (
            tc.tile_pool(name="const", bufs=1) as cp,
            tc.tile_pool(name="wide", bufs=2) as wp,
            tc.tile_pool(name="gath", bufs=2) as gp,
            tc.tile_pool(name="work", bufs=2) as wk,
            tc.tile_pool(name="psum", bufs=2, space="PSUM") as pp,
        ):
            # ---- constants ----
            iota_sb = cp.tile([P, 1, P], F16)
            nc.sync.dma_start(out=iota_sb[:, 0, :], in_=iotap[:, :])
            ident_sb = cp.tile([P, P], F16)
            nc.sync.dma_start(out=ident_sb[:], in_=identp[:, :])
            conf_sb = cp.tile([P, 4], F32)
            nc.sync.dma_start(out=conf_sb[:], in_=confp[:, :])
            bias_sb = cp.tile([P, d_out], F32)
            nc.sync.dma_start(out=bias_sb[:], in_=biasp[:, :])
            wt_sb = cp.tile([P, KC, d_out], F16)
            nc.sync.dma_start(
                out=wt_sb[:], in_=wtp[:, :].rearrange("(kc p) o -> p kc o", p=P)
            )
            cloc_sb = cp.tile([P, CH_TOT], F16)
            nc.sync.dma_start(out=cloc_sb[:], in_=clocp[:, :])
            idx_sb = cp.tile([P, L_TOT // 16], I16)
            nc.sync.dma_start(out=idx_sb[:], in_=idxp[:, :])
            att_sb = cp.tile([P, 3, CH_TOT], F32)
            nc.sync.dma_start(
                out=att_sb[:], in_=attrp[:, :, :].rearrange("t p c -> p t c")
            )

            # ---- dis = where(deg>0, 1/sqrt(deg), 0) ----
            def masked_rsqrt(dst, deg_sb, n):
                safe = cp.tile(list(deg_sb.shape), F32, tag=f"rs_safe{n}")
                nc.vector.tensor_scalar_max(safe[:], deg_sb[:], 1.0)
                nc.scalar.sqrt(safe[:], safe[:])
                nc.vector.reciprocal(safe[:], safe[:])
                mask = cp.tile(list(deg_sb.shape), F32, tag=f"rs_mask{n}")
                nc.vector.tensor_scalar(
                    mask[:], deg_sb[:], 0.0, None, mybir.AluOpType.is_gt
                )
                nc.vector.tensor_tensor(
                    dst[:], safe[:], mask[:], mybir.AluOpType.mult
                )

            degf_sb = cp.tile([P, NBP], F32)
            nc.sync.dma_start(out=degf_sb[:], in_=degt[:, :])
            disf_sb = cp.tile([P, NBP], F32)
            masked_rsqrt(disf_sb, degf_sb, "f")

            dego_sb = cp.tile([P, BPC], F32)
            nc.sync.dma_start(out=dego_sb[:], in_=dego[:, :])
            diso_sb = cp.tile([P, BPC], F32)
            masked_rsqrt(diso_sb, dego_sb, "o")

            # ---- dis_wide[r, :] = dis[r] (f16, 256B rows for dma_gather) ----
            for g in range(plan["n_cores"] if "no_diswide" not in debug_mode else 0):
                wt_tile = wp.tile([P, BPC, P], F16, tag="diswide")
                nc.vector.tensor_copy(
                    out=wt_tile[:],
                    in_=disf_sb[:, g * BPC : (g + 1) * BPC, None].to_broadcast(
                        [P, BPC, P]
                    ),
                )
                nc.sync.dma_start(
                    out=dis_wide[g * BPC * P : (g + 1) * BPC * P, :].rearrange(
                        "(B p) j -> p B j", p=P
                    ),
                    in_=wt_tile[:],
                )

            # ---- w = sigmoid(attr @ conf_w + conf_b) over all core edges ----
            acc = cp.tile([P, CH_TOT], F32, tag="wacc")
            tmp = cp.tile([P, CH_TOT], F32, tag="wtmp")
            nc.vector.tensor_scalar_mul(acc[:], att_sb[:, 0, :], conf_sb[:, 0:1])
            nc.vector.tensor_scalar_mul(tmp[:], att_sb[:, 1, :], conf_sb[:, 1:2])
            nc.vector.tensor_tensor(acc[:], acc[:], tmp[:], mybir.AluOpType.add)
            nc.vector.tensor_scalar_mul(tmp[:], att_sb[:, 2, :], conf_sb[:, 2:3])
            nc.vector.tensor_tensor(acc[:], acc[:], tmp[:], mybir.AluOpType.add)
            w_sb = cp.tile([P, CH_TOT], F16, tag="wsig")
            nc.scalar.activation(
                w_sb[:], acc[:], mybir.ActivationFunctionType.Sigmoid,
                bias=conf_sb[:, 3:4], scale=1.0,
            )

            # ---- per destination-block pipeline ----
            for b in range(BPC):
                cb = int(C[b])
                clo, chi = int(C_LO[b]), int(C_HI[b])
                co = int(choff[b])
                y_sb = wk.tile([P, d_out], F32, tag="ysb")
                if cb == 0:
                    nc.vector.tensor_copy(out=y_sb[:], in_=bias_sb[:])
                    nc.sync.dma_start(
                        out=yout[b * P : (b + 1) * P, :], in_=y_sb[:]
                    )
                    continue

                xg = gp.tile([P, CMAX, d_in], F16, tag="xg")
                dr = gp.tile([P, CMAX, P], F16, tag="dr")

                # HW dma_gather caps at 1024 indices (8 chunks) per
                # instruction (Q7 per-core batch limit) - split.
                MAXCH = 8

                def emit_gathers(dst, src_ap, dst0, idx0, nch, elem):
                    for s in range(0, nch, MAXCH):
                        n = min(MAXCH, nch - s)
                        nc.gpsimd.dma_gather(
                            dst[:, dst0 + s : dst0 + s + n, :], src_ap,
                            idx_sb[:, (idx0 + s) * 8 : (idx0 + s + n) * 8],
                            n * P, n * P, elem,
                        )

                if "no_gather_x" in debug_mode:
                    nc.vector.memset(xg[:, :cb, :], 0.25)
                else:
                    emit_gathers(xg, x16[:, :], 0, co, clo, d_in)
                    emit_gathers(xg, x16[HALF:, :], clo, co + clo, chi, d_in)
                if "no_gather_d" in debug_mode:
                    nc.vector.memset(dr[:, :cb, :], 0.5)
                else:
                    emit_gathers(dr, dis_wide[:, :], 0, co, clo, P)
                    emit_gathers(dr, dis_wide[HALF:, :], clo, co + clo, chi, P)

                if "no_pe" in debug_mode:
                    nc.vector.tensor_copy(out=y_sb[:], in_=bias_sb[:])
                    nc.sync.dma_start(
                        out=yout[b * P : (b + 1) * P, :], in_=y_sb[:]
                    )
                    continue

                # combined edge scalar = w_e * dis[row_e]
                comb = wk.tile([P, CMAX], F16, tag="comb")
                nc.vector.tensor_tensor(
                    comb[:, :cb], w_sb[:, co : co + cb], dr[:, :cb, 0],
                    mybir.AluOpType.mult,
                )
                # Wsel[e, c, d] = comb * (col_local == d)
                wsel = wk.tile([P, CMAX, P], F16, tag="wsel")
                nc.vector.tensor_tensor(
                    wsel[:, :cb, :],
                    cloc_sb[:, co : co + cb, None].to_broadcast([P, cb, P]),
                    iota_sb[:, 0:1, :].to_broadcast([P, cb, P]),
                    mybir.AluOpType.is_equal,
                )
                nc.vector.tensor_tensor(
                    wsel[:, :cb, :],
                    wsel[:, :cb, :],
                    comb[:, :cb, None].to_broadcast([P, cb, P]),
                    mybir.AluOpType.mult,
                )

                # SpMM: psum[d, k] += sum_e Wsel[e, d] * Xg[e, k]
                ps = pp.tile([P, d_in], F32, tag="spmm")
                for cc in range(cb):
                    nc.tensor.matmul(
                        ps[:], lhsT=wsel[:, cc, :], rhs=xg[:, cc, :],
                        start=(cc == 0), stop=(cc == cb - 1),
                    )
                spmm_sb = wk.tile([P, d_in], F16, tag="spmmsb")
                nc.vector.tensor_copy(out=spmm_sb[:], in_=ps[:])

                # transpose [d, k] -> [k, d] per 128-chunk
                pst = pp.tile([P, KC, P], F16, tag="tr")
                for kc in range(KC):
                    nc.tensor.transpose(
                        pst[:, kc, :], spmm_sb[:, kc * P : (kc + 1) * P], ident_sb[:]
                    )
                outT = wk.tile([P, KC, P], F16, tag="outT")
                nc.vector.tensor_copy(out=outT[:], in_=pst[:])

                # linear: y[d, o] = sum_k outT[k, d] * WT[k, o]
                py = pp.tile([P, d_out], F32, tag="ylin")
                for kc in range(KC):
                    nc.tensor.matmul(
                        py[:], lhsT=outT[:, kc, :], rhs=wt_sb[:, kc, :],
                        start=(kc == 0), stop=(kc == KC - 1),
                    )
                nc.vector.tensor_scalar_mul(y_sb[:], py[:], diso_sb[:, b : b + 1])
                nc.vector.tensor_tensor(
                    y_sb[:], y_sb[:], bias_sb[:], mybir.AluOpType.add
                )
                nc.sync.dma_start(out=yout[b * P : (b + 1) * P, :], in_=y_sb[:])

    nc.compile()
    return nc


def make_in_maps(plan, core_arrays, x, lin_w, lin_b, conf_w, conf_b, edge_index):
    n_nodes = plan["n_nodes"]
    NBP, BPC, NPAD = plan["NBP"], plan["BPC"], plan["NPAD"]
    row = np.asarray(edge_index[0], dtype=np.int64)

    x16 = np.ascontiguousarray(np.asarray(x, np.float32).astype(np.float16))
    deg = np.bincount(row, minlength=NPAD).astype(np.float32)
    degt = np.ascontiguousarray(deg.reshape(NBP, P).T)  # [128, NBP]
    conf_row = np.concatenate(
        [np.asarray(conf_w, np.float32).reshape(-1), np.asarray(conf_b, np.float32).reshape(-1)]
    ).astype(np.float32)
    confr = np.tile(conf_row, (P, 1))
    wt = np.ascontiguousarray(np.asarray(lin_w, np.float32).T.astype(np.float16))
    biasr = np.tile(np.asarray(lin_b, np.float32).reshape(1, -1), (P, 1)).astype(np.float32)
    iota = np.tile(np.arange(P, dtype=np.float16), (P, 1))
    ident = np.eye(P, dtype=np.float16)

    in_maps = []
    for c in range(plan["n_cores"]):
        arr = core_arrays[c]
        in_maps.append({
            "x16": x16,
            "degt": degt,
            "dego": np.ascontiguousarray(degt[:, c * BPC : (c + 1) * BPC]),
            "idx": arr["idx"],
            "cloc": arr["cloc"],
            "attr": arr["attr"],
            "confr": confr,
            "wt": wt,
            "biasr": biasr,
            "iota": iota,
            "ident": ident,
        })
    return in_maps


def _run(x, edge_index, edge_attr, lin_w, lin_b, conf_w, conf_b, **run_kwargs):
    n_nodes, d_in = x.shape
    d_out = lin_w.shape[0]
    plan = make_plan(edge_index, n_nodes)
    core_arrays = make_core_arrays(plan, edge_index, edge_attr)
    nc = build_program(plan, d_in, d_out)
    in_maps = make_in_maps(plan, core_arrays, x, lin_w, lin_b, conf_w, conf_b, edge_index)
    res = run_bass_kernel_spmd(nc, in_maps, list(range(plan["n_cores"])), **run_kwargs)
    ys = [np.asarray(res.results[c]["y"], np.float32) for c in range(plan["n_cores"])]
    y = np.concatenate(ys, axis=0)[:n_nodes]
    return y, res


def kernel(x, edge_index, edge_attr, lin_w, lin_b, conf_w, conf_b):
    y, _ = _run(x, edge_index, edge_attr, lin_w, lin_b, conf_w, conf_b)
    return y.astype(np.float32)


# revision 19
# speedup vs baseline: 1.3059x; 1.3059x over previous
"""Trainium2 Bass kernel for ConfidenceGCNConv message passing.

Math (reference):
    w_e   = sigmoid(edge_attr @ conf_w.T + conf_b)            # [E]
    deg   = bincount(row); dis = where(deg>0, rsqrt(deg), 0)  # [N]
    out[c] = sum_{e: col_e=c} dis[row_e]*dis[col_e]*w_e * x[row_e]
    y     = out @ lin_w.T + lin_b

Device strategy (8 cores, destination-partitioned):
  - Host: partition edges by destination block (128 dests / block, 49 blocks
    per core), split per block into row<HALF / row>=HALF halves (int16 gather
    index limit), pad each half to whole 128-lane chunks. Pure index work.
  - Device per core, per destination block b:
      * dma_gather x16[row_e] rows (fp16, 1KB each) -> Xg [128e, C, 512]
      * dma_gather dis_wide[row_e] (256B rows, col 0 = dis[row]) -> Dr
      * Wsel[e, d] = w_e * dis[row_e] * (col_local_e == d)   (DVE iota-compare)
      * PSUM spmm[128d, 512k] += sum_c Wsel_c^T @ Xg_c       (PE, contraction=edges)
      * transpose spmm -> outT [128k, 4, 128d]               (PE identity trick)
      * PSUM y[128d, 512o] = sum_kc outT_kc^T @ WT_kc        (PE)
      * y_sb = y * dis[dest] + bias                          (scale commutes with @W)
  - w_e = sigmoid(...) and dis = masked 1/sqrt(deg) computed on device.
    dis_wide (replicated 128-wide dis table for 256B-aligned gathers) is
    built on device once per core.
"""

import sys

for _p in ("/opt/trn_rl_repo",):
    if _p not in sys.path:
        sys.path.insert(0, _p)

import numpy as np

import concourse.bass as bass
import concourse.mybir as mybir
import concourse.tile as tile
from concourse import bacc
from concourse.bass_utils import run_bass_kernel_spmd

P = 128
NCORES = 8
F16 = mybir.dt.float16
F32 = mybir.dt.float32
I16 = mybir.dt.int16


def _cdiv(a, b):
    return (a + b - 1) // b


def make_plan(edge_index, n_nodes, n_cores=NCORES):
    """Host-side integer/index preprocessing: edge partition + padded layout.

    Returns a dict with static program metadata (shared across cores) and the
    per-edge placement (core, lane) used to build per-core input arrays.
    """
    row = np.asarray(edge_index[0], dtype=np.int64)
    col = np.asarray(edge_index[1], dtype=np.int64)
    E = row.shape[0]

    NB = _cdiv(n_nodes, P)           # blocks covering all nodes
    BPC = _cdiv(NB, n_cores)         # blocks per core
    NBP = BPC * n_cores              # padded total blocks
    NPAD = NBP * P                   # padded node count (dest side)
    HALF = n_nodes // 2
    assert HALF <= 32768 and (n_nodes - HALF) <= 32768

    gb = col // P                    # global destination block
    core = gb // BPC
    b = gb - core * BPC              # block local to core
    half = (row >= HALF).astype(np.int64)
    key = (core * BPC + b) * 2 + half

    order = np.argsort(key, kind="stable")
    counts = np.bincount(key, minlength=2 * NBP)
    cnt3 = counts.reshape(n_cores, BPC, 2)
    chunks_needed = _cdiv(cnt3, P)
    C_LO = chunks_needed[:, :, 0].max(axis=0)  # [BPC], shared across cores
    C_HI = chunks_needed[:, :, 1].max(axis=0)
    C = C_LO + C_HI
    choff = np.zeros(BPC + 1, dtype=np.int64)
    np.cumsum(C, out=choff[1:])
    CH_TOT = int(choff[-1])
    L_TOT = CH_TOT * P

    # lane base per key (core-independent): lo at choff[b]*128, hi after lo
    lane_lo = choff[:BPC] * P
    lane_hi = lane_lo + C_LO * P
    lane_base_k = np.zeros(2 * NBP, dtype=np.int64)
    for c in range(n_cores):
        lane_base_k[(c * BPC + np.arange(BPC)) * 2 + 0] = lane_lo
        lane_base_k[(c * BPC + np.arange(BPC)) * 2 + 1] = lane_hi

    sorted_key = key[order]
    starts = np.zeros(2 * NBP, dtype=np.int64)
    np.cumsum(counts[:-1], out=starts[1:])
    pos_in_grp = np.arange(E, dtype=np.int64) - starts[sorted_key]
    lane = lane_base_k[sorted_key] + pos_in_grp
    edge_core = sorted_key // (2 * BPC)

    return dict(
        n_nodes=n_nodes, E=E, NB=NB, BPC=BPC, NBP=NBP, NPAD=NPAD, HALF=HALF,
        C_LO=C_LO.astype(int), C_HI=C_HI.astype(int), C=C.astype(int),
        choff=choff.astype(int), CH_TOT=CH_TOT, L_TOT=L_TOT,
        order=order, lane=lane, edge_core=edge_core,
        n_cores=n_cores,
    )


def make_core_arrays(plan, edge_index, edge_attr):
    """Per-core padded gather-index / col-local / edge-attr arrays."""
    row = np.asarray(edge_index[0], dtype=np.int64)
    col = np.asarray(edge_index[1], dtype=np.int64)
    HALF = plan["HALF"]
    CH_TOT = plan["CH_TOT"]
    L_TOT = plan["L_TOT"]
    out = []
    for c in range(plan["n_cores"]):
        sel = plan["edge_core"] == c
        e_sel = plan["order"][sel]
        lanes = plan["lane"][sel]
        rows = row[e_sel]
        idxval = np.where(rows < HALF, rows, rows - HALF).astype(np.int16)

        idx2d = np.zeros((16, L_TOT // 16), dtype=np.int16)
        idx2d[lanes % 16, lanes // 16] = idxval
        # HW reads the 16-partition wrap replicated across all 8 Q7 core
        # groups (sim only reads partitions 0-15; HW cores read their own).
        idx2d = np.tile(idx2d, (8, 1))

        cloc = np.full((P, CH_TOT), float(P), dtype=np.float16)  # 128 = sentinel
        cloc[lanes % P, lanes // P] = (col[e_sel] % P).astype(np.float16)

        attr3 = np.zeros((3, P, CH_TOT), dtype=np.float32)
        attr3[:, lanes % P, lanes // P] = np.asarray(edge_attr, np.float32)[e_sel].T

        out.append(dict(idx=idx2d, cloc=cloc, attr=attr3))
    return out


def build_program(plan, d_in, d_out, debug_mode=()):
    """Build the (SPMD-shared) Bass program.

    debug_mode: set of strings to disable pieces for HW bisection:
      'no_gather_x'  - memset xg instead of dma_gather from x16
      'no_gather_d'  - memset dr instead of dma_gather from dis_wide
      'no_diswide'   - skip dis_wide build
      'no_pe'        - skip matmul/transpose; y = bias only
    """
    debug_mode = set(debug_mode)
    n_nodes = plan["n_nodes"]
    NPAD, NBP, BPC = plan["NPAD"], plan["NBP"], plan["BPC"]
    HALF = plan["HALF"]
    C_LO, C_HI, C, choff = plan["C_LO"], plan["C_HI"], plan["C"], plan["choff"]
    CH_TOT, L_TOT = plan["CH_TOT"], plan["L_TOT"]
    CMAX = int(max(C))
    KC = d_in // P          # feature chunks (4)
    assert d_in % P == 0 and d_out % P == 0

    nc = bacc.Bacc("TRN2")

    x16 = nc.declare_dram_parameter("x16", [NPAD, d_in], F16, isOutput=False)
    degt = nc.declare_dram_parameter("degt", [P, NBP], F32, isOutput=False)
    dego = nc.declare_dram_parameter("dego", [P, BPC], F32, isOutput=False)
    idxp = nc.declare_dram_parameter("idx", [P, L_TOT // 16], I16, isOutput=False)
    clocp = nc.declare_dram_parameter("cloc", [P, CH_TOT], F16, isOutput=False)
    attrp = nc.declare_dram_parameter("attr", [3, P, CH_TOT], F32, isOutput=False)
    confp = nc.declare_dram_parameter("confr", [P, 4], F32, isOutput=False)
    wtp = nc.declare_dram_parameter("wt", [d_in, d_out], F16, isOutput=False)
    biasp = nc.declare_dram_parameter("biasr", [P, d_out], F32, isOutput=False)
    iotap = nc.declare_dram_parameter("iota", [P, P], F16, isOutput=False)
    identp = nc.declare_dram_parameter("ident", [P, P], F16, isOutput=False)
    yout = nc.declare_dram_parameter("y", [BPC * P, d_out], F32, isOutput=True)

    zt_dram = nc.dram_tensor("z", [NPAD, d_in], F16)

    with tile.TileContext(nc) as tc:
        with (
            tc.tile_pool(name="const", bufs=1) as cp,
            tc.tile_pool(name="wide", bufs=2) as wp,
            tc.tile_pool(name="gath", bufs=2) as gp,
            tc.tile_pool(name="work", bufs=2) as wk,
            tc.tile_pool(name="psum", bufs=2, space="PSUM") as pp,
        ):
            # ---- constants ----
            iota_sb = cp.tile([P, 1, P], F16)
            nc.sync.dma_start(out=iota_sb[:, 0, :], in_=iotap[:, :])
            ident_sb = cp.tile([P, P], F16)
            nc.sync.dma_start(out=ident_sb[:], in_=identp[:, :])
            conf_sb = cp.tile([P, 4], F32)
            nc.sync.dma_start(out=conf_sb[:], in_=confp[:, :])
            bias_sb = cp.tile([P, d_out], F32)
            nc.sync.dma_start(out=bias_sb[:], in_=biasp[:, :])
            wt_sb = cp.tile([P, KC, d_out], F16)
            nc.sync.dma_start(
                out=wt_sb[:], in_=wtp[:, :].rearrange("(kc p) o -> p kc o", p=P)
            )
            cloc_sb = cp.tile([P, CH_TOT], F16)
            nc.sync.dma_start(out=cloc_sb[:], in_=clocp[:, :])
            idx_sb = cp.tile([P, L_TOT // 16], I16)
            nc.sync.dma_start(out=idx_sb[:], in_=idxp[:, :])
            att_sb = cp.tile([P, 3, CH_TOT], F32)
            nc.sync.dma_start(
                out=att_sb[:], in_=attrp[:, :, :].rearrange("t p c -> p t c")
            )

            # ---- dis = where(deg>0, 1/sqrt(deg), 0) ----
            def masked_rsqrt(dst, deg_sb, n):
                safe = cp.tile(list(deg_sb.shape), F32, tag=f"rs_safe{n}")
                nc.vector.tensor_scalar_max(safe[:], deg_sb[:], 1.0)
                nc.scalar.sqrt(safe[:], safe[:])
                nc.vector.reciprocal(safe[:], safe[:])
                mask = cp.tile(list(deg_sb.shape), F32, tag=f"rs_mask{n}")
                nc.vector.tensor_scalar(
                    mask[:], deg_sb[:], 0.0, None, mybir.AluOpType.is_gt
                )
                nc.vector.tensor_tensor(
                    dst[:], safe[:], mask[:], mybir.AluOpType.mult
                )

            degf_sb = cp.tile([P, NBP], F32)
            nc.sync.dma_start(out=degf_sb[:], in_=degt[:, :])
            disf_sb = cp.tile([P, NBP], F32)
            masked_rsqrt(disf_sb, degf_sb, "f")

            dego_sb = cp.tile([P, BPC], F32)
            nc.sync.dma_start(out=dego_sb[:], in_=dego[:, :])
            diso_sb = cp.tile([P, BPC], F32)
            masked_rsqrt(diso_sb, dego_sb, "o")

            # ---- z[r, :] = dis[r] * x[r, :]  (prescaled gather table, f16) ----
            # Row-block layout: partition = node row % 128. Big slabs keep
            # the DMAs on the fast HWDGE path (no Q7 involvement).
            SLAB = 8
            x16_t = x16[:, :].rearrange("(o p) k -> p o k", p=P)
            z_t = zt_dram[:, :].rearrange("(o p) k -> p o k", p=P)
            if "no_diswide" not in debug_mode:
                for s in range(0, NBP, SLAB):
                    xt = wp.tile([P, SLAB, d_in], F16, tag="zslab")
                    nc.sync.dma_start(out=xt[:], in_=x16_t[:, s : s + SLAB, :])
                    nc.vector.tensor_tensor(
                        xt[:], xt[:],
                        disf_sb[:, s : s + SLAB, None].to_broadcast([P, SLAB, d_in]),
                        mybir.AluOpType.mult,
                    )
                    nc.sync.dma_start(out=z_t[:, s : s + SLAB, :], in_=xt[:])

            # ---- w = sigmoid(attr @ conf_w + conf_b) over all core edges ----
            acc = cp.tile([P, CH_TOT], F32, tag="wacc")
            tmp = cp.tile([P, CH_TOT], F32, tag="wtmp")
            nc.vector.tensor_scalar_mul(acc[:], att_sb[:, 0, :], conf_sb[:, 0:1])
            nc.vector.tensor_scalar_mul(tmp[:], att_sb[:, 1, :], conf_sb[:, 1:2])
            nc.vector.tensor_tensor(acc[:], acc[:], tmp[:], mybir.AluOpType.add)
            nc.vector.tensor_scalar_mul(tmp[:], att_sb[:, 2, :], conf_sb[:, 2:3])
            nc.vector.tensor_tensor(acc[:], acc[:], tmp[:], mybir.AluOpType.add)
            w_sb = cp.tile([P, CH_TOT], F16, tag="wsig")
            nc.scalar.activation(
                w_sb[:], acc[:], mybir.ActivationFunctionType.Sigmoid,
                bias=conf_sb[:, 3:4], scale=1.0,
            )

            # ---- per destination-block pipeline ----
            for b in range(BPC):
                cb = int(C[b])
                clo, chi = int(C_LO[b]), int(C_HI[b])
                co = int(choff[b])
                y_sb = wk.tile([P, d_out], F32, tag="ysb")
                if cb == 0:
                    nc.vector.tensor_copy(out=y_sb[:], in_=bias_sb[:])
                    nc.sync.dma_start(
                        out=yout[b * P : (b + 1) * P, :], in_=y_sb[:]
                    )
                    continue

                xg = gp.tile([P, CMAX, d_in], F16, tag="xg")

                # HW dma_gather caps at 1024 indices (8 chunks) per
                # instruction (Q7 per-core batch limit) - split.
                MAXCH = 8

                def emit_gathers(dst, src_ap, dst0, idx0, nch, elem):
                    for s in range(0, nch, MAXCH):
                        n = min(MAXCH, nch - s)
                        nc.gpsimd.dma_gather(
                            dst[:, dst0 + s : dst0 + s + n, :], src_ap,
                            idx_sb[:, (idx0 + s) * 8 : (idx0 + s + n) * 8],
                            n * P, n * P, elem,
                        )

                if "no_gather_x" in debug_mode:
                    nc.vector.memset(xg[:, :cb, :], 0.25)
                else:
                    emit_gathers(xg, zt_dram[:, :], 0, co, clo, d_in)
                    emit_gathers(xg, zt_dram[HALF:, :], clo, co + clo, chi, d_in)

                if "no_pe" in debug_mode:
                    nc.vector.tensor_copy(out=y_sb[:], in_=bias_sb[:])
                    nc.sync.dma_start(
                        out=yout[b * P : (b + 1) * P, :], in_=y_sb[:]
                    )
                    continue

                # Wsel[e, c, d] = w_e * (col_local == d)
                wsel = wk.tile([P, CMAX, P], F16, tag="wsel")
                nc.vector.tensor_tensor(
                    wsel[:, :cb, :],
                    cloc_sb[:, co : co + cb, None].to_broadcast([P, cb, P]),
                    iota_sb[:, 0:1, :].to_broadcast([P, cb, P]),
                    mybir.AluOpType.is_equal,
                )
                nc.vector.tensor_tensor(
                    wsel[:, :cb, :],
                    wsel[:, :cb, :],
                    w_sb[:, co : co + cb, None].to_broadcast([P, cb, P]),
                    mybir.AluOpType.mult,
                )

                # SpMM: psum[d, k] += sum_e Wsel[e, d] * Xg[e, k]
                ps = pp.tile([P, d_in], F32, tag="spmm")
                for cc in range(cb):
                    nc.tensor.matmul(
                        ps[:], lhsT=wsel[:, cc, :], rhs=xg[:, cc, :],
                        start=(cc == 0), stop=(cc == cb - 1),
                    )
                spmm_sb = wk.tile([P, d_in], F16, tag="spmmsb")
                nc.vector.tensor_copy(out=spmm_sb[:], in_=ps[:])

                # transpose [d, k] -> [k, d] per 128-chunk
                pst = pp.tile([P, KC, P], F16, tag="tr")
                for kc in range(KC):
                    nc.tensor.transpose(
                        pst[:, kc, :], spmm_sb[:, kc * P : (kc + 1) * P], ident_sb[:]
                    )
                outT = wk.tile([P, KC, P], F16, tag="outT")
                nc.vector.tensor_copy(out=outT[:], in_=pst[:])

                # linear: y[d, o] = sum_k outT[k, d] * WT[k, o]
                py = pp.tile([P, d_out], F32, tag="ylin")
                for kc in range(KC):
                    nc.tensor.matmul(
                        py[:], lhsT=outT[:, kc, :], rhs=wt_sb[:, kc, :],
                        start=(kc == 0), stop=(kc == KC - 1),
                    )
                nc.vector.tensor_scalar_mul(y_sb[:], py[:], diso_sb[:, b : b + 1])
                nc.vector.tensor_tensor(
                    y_sb[:], y_sb[:], bias_sb[:], mybir.AluOpType.add
                )
                nc.sync.dma_start(out=yout[b * P : (b + 1) * P, :], in_=y_sb[:])

    nc.compile()
    return nc


def make_in_maps(plan, core_arrays, x, lin_w, lin_b, conf_w, conf_b, edge_index):
    n_nodes = plan["n_nodes"]
    NBP, BPC, NPAD = plan["NBP"], plan["BPC"], plan["NPAD"]
    row = np.asarray(edge_index[0], dtype=np.int64)

    x16 = np.zeros((NPAD, x.shape[1]), dtype=np.float16)
    x16[: x.shape[0]] = np.asarray(x, np.float32).astype(np.float16)
    deg = np.bincount(row, minlength=NPAD).astype(np.float32)
    degt = np.ascontiguousarray(deg.reshape(NBP, P).T)  # [128, NBP]
    conf_row = np.concatenate(
        [np.asarray(conf_w, np.float32).reshape(-1), np.asarray(conf_b, np.float32).reshape(-1)]
    ).astype(np.float32)
    confr = np.tile(conf_row, (P, 1))
    wt = np.ascontiguousarray(np.asarray(lin_w, np.float32).T.astype(np.float16))
    biasr = np.tile(np.asarray(lin_b, np.float32).reshape(1, -1), (P, 1)).astype(np.float32)
    iota = np.tile(np.arange(P, dtype=np.float16), (P, 1))
    ident = np.eye(P, dtype=np.float16)

    in_maps = []
    for c in range(plan["n_cores"]):
        arr = core_arrays[c]
        in_maps.append({
            "x16": x16,
            "degt": degt,
            "dego": np.ascontiguousarray(degt[:, c * BPC : (c + 1) * BPC]),
            "idx": arr["idx"],
            "cloc": arr["cloc"],
            "attr": arr["attr"],
            "confr": confr,
            "wt": wt,
            "biasr": biasr,
            "iota": iota,
            "ident": ident,
        })
    return in_maps


def _run(x, edge_index, edge_attr, lin_w, lin_b, conf_w, conf_b, **run_kwargs):
    n_nodes, d_in = x.shape
    d_out = lin_w.shape[0]
    plan = make_plan(edge_index, n_nodes)
    core_arrays = make_core_arrays(plan, edge_index, edge_attr)
    nc = build_program(plan, d_in, d_out)
    in_maps = make_in_maps(plan, core_arrays, x, lin_w, lin_b, conf_w, conf_b, edge_index)
    res = run_bass_kernel_spmd(nc, in_maps, list(range(plan["n_cores"])), **run_kwargs)
    ys = [np.asarray(res.results[c]["y"], np.float32) for c in range(plan["n_cores"])]
    y = np.concatenate(ys, axis=0)[:n_nodes]
    return y, res


def kernel(x, edge_index, edge_attr, lin_w, lin_b, conf_w, conf_b):
    y, _ = _run(x, edge_index, edge_attr, lin_w, lin_b, conf_w, conf_b)
    return y.astype(np.float32)


# revision 26
# speedup vs baseline: 1.8194x; 1.3932x over previous
"""Trainium2 Bass kernel for ConfidenceGCNConv message passing.

Math (reference):
    w_e   = sigmoid(edge_attr @ conf_w.T + conf_b)            # [E]
    deg   = bincount(row); dis = where(deg>0, rsqrt(deg), 0)  # [N]
    out[c] = sum_{e: col_e=c} dis[row_e]*dis[col_e]*w_e * x[row_e]
    y     = out @ lin_w.T + lin_b

Device strategy (8 cores, destination-partitioned):
  - Host: partition edges by destination block (128 dests / block, 49 blocks
    per core), split per block into row<HALF / row>=HALF halves (int16 gather
    index limit), pad each half to whole 128-lane chunks. Pure index work.
  - Device per core, per destination block b:
      * dma_gather x16[row_e] rows (fp16, 1KB each) -> Xg [128e, C, 512]
      * dma_gather dis_wide[row_e] (256B rows, col 0 = dis[row]) -> Dr
      * Wsel[e, d] = w_e * dis[row_e] * (col_local_e == d)   (DVE iota-compare)
      * PSUM spmm[128d, 512k] += sum_c Wsel_c^T @ Xg_c       (PE, contraction=edges)
      * transpose spmm -> outT [128k, 4, 128d]               (PE identity trick)
      * PSUM y[128d, 512o] = sum_kc outT_kc^T @ WT_kc        (PE)
      * y_sb = y * dis[dest] + bias                          (scale commutes with @W)
  - w_e = sigmoid(...) and dis = masked 1/sqrt(deg) computed on device.
    dis_wide (replicated 128-wide dis table for 256B-aligned gathers) is
    built on device once per core.
"""

import sys

for _p in ("/opt/trn_rl_repo",):
    if _p not in sys.path:
        sys.path.insert(0, _p)

import numpy as np

import concourse.bass as bass
import concourse.mybir as mybir
import concourse.tile as tile
from concourse import bacc
from concourse.bass_utils import run_bass_kernel_spmd

P = 128
NCORES = 8
F16 = mybir.dt.float16
F32 = mybir.dt.float32
I16 = mybir.dt.int16


def _cdiv(a, b):
    return (a + b - 1) // b


def make_plan(edge_index, n_nodes, n_cores=NCORES):
    """Host-side integer/index preprocessing: edge partition + padded layout.

    Returns a dict with static program metadata (shared across cores) and the
    per-edge placement (core, lane) used to build per-core input arrays.
    """
    row = np.asarray(edge_index[0], dtype=np.int64)
    col = np.asarray(edge_index[1], dtype=np.int64)
    E = row.shape[0]

    NB = _cdiv(n_nodes, P)           # blocks covering all nodes
    BPC = _cdiv(NB, n_cores)         # blocks per core
    NBP = BPC * n_cores              # padded total blocks
    NPAD = NBP * P                   # padded node count (dest side)
    HALF = n_nodes // 2
    assert HALF <= 32768 and (n_nodes - HALF) <= 32768

    gb = col // P                    # global destination block
    core = gb // BPC
    b = gb - core * BPC              # block local to core
    half = (row >= HALF).astype(np.int64)
    key = (core * BPC + b) * 2 + half

    order = np.argsort(key, kind="stable")
    counts = np.bincount(key, minlength=2 * NBP)
    cnt3 = counts.reshape(n_cores, BPC, 2)
    chunks_needed = _cdiv(cnt3, P)
    C_LO = chunks_needed[:, :, 0].max(axis=0)  # [BPC], shared across cores
    C_HI = chunks_needed[:, :, 1].max(axis=0)
    C = C_LO + C_HI
    choff = np.zeros(BPC + 1, dtype=np.int64)
    np.cumsum(C, out=choff[1:])
    CH_TOT = int(choff[-1])
    L_TOT = CH_TOT * P

    # lane base per key (core-independent): lo at choff[b]*128, hi after lo
    lane_lo = choff[:BPC] * P
    lane_hi = lane_lo + C_LO * P
    lane_base_k = np.zeros(2 * NBP, dtype=np.int64)
    for c in range(n_cores):
        lane_base_k[(c * BPC + np.arange(BPC)) * 2 + 0] = lane_lo
        lane_base_k[(c * BPC + np.arange(BPC)) * 2 + 1] = lane_hi

    sorted_key = key[order]
    starts = np.zeros(2 * NBP, dtype=np.int64)
    np.cumsum(counts[:-1], out=starts[1:])
    pos_in_grp = np.arange(E, dtype=np.int64) - starts[sorted_key]
    lane = lane_base_k[sorted_key] + pos_in_grp
    edge_core = sorted_key // (2 * BPC)

    return dict(
        n_nodes=n_nodes, E=E, NB=NB, BPC=BPC, NBP=NBP, NPAD=NPAD, HALF=HALF,
        C_LO=C_LO.astype(int), C_HI=C_HI.astype(int), C=C.astype(int),
        choff=choff.astype(int), CH_TOT=CH_TOT, L_TOT=L_TOT,
        order=order, lane=lane, edge_core=edge_core,
        n_cores=n_cores,
    )


def make_core_arrays(plan, edge_index, edge_attr):
    """Per-core padded gather-index / col-local / edge-attr arrays."""
    row = np.asarray(edge_index[0], dtype=np.int64)
    col = np.asarray(edge_index[1], dtype=np.int64)
    HALF = plan["HALF"]
    CH_TOT = plan["CH_TOT"]
    L_TOT = plan["L_TOT"]
    out = []
    for c in range(plan["n_cores"]):
        sel = plan["edge_core"] == c
        e_sel = plan["order"][sel]
        lanes = plan["lane"][sel]
        rows = row[e_sel]
        idxval = np.where(rows < HALF, rows, rows - HALF).astype(np.int16)

        idx2d = np.zeros((16, L_TOT // 16), dtype=np.int16)
        idx2d[lanes % 16, lanes // 16] = idxval
        # HW reads the 16-partition wrap replicated across all 8 Q7 core
        # groups (sim only reads partitions 0-15; HW cores read their own).
        idx2d = np.tile(idx2d, (8, 1))

        cloc = np.full((P, CH_TOT), float(P), dtype=np.float16)  # 128 = sentinel
        cloc[lanes % P, lanes // P] = (col[e_sel] % P).astype(np.float16)

        attr3 = np.zeros((3, P, CH_TOT), dtype=np.float32)
        attr3[:, lanes % P, lanes // P] = np.asarray(edge_attr, np.float32)[e_sel].T

        out.append(dict(idx=idx2d, cloc=cloc, attr=attr3, rows=rows, lanes=lanes))
    return out


def build_program(plan, d_in, d_out, debug_mode=()):
    """Build the (SPMD-shared) Bass program.

    debug_mode: set of strings to disable pieces for HW bisection:
      'no_gather_x'  - memset xg instead of dma_gather from x16
      'no_gather_d'  - memset dr instead of dma_gather from dis_wide
      'no_diswide'   - skip dis_wide build
      'no_pe'        - skip matmul/transpose; y = bias only
    """
    debug_mode = set(debug_mode)
    n_nodes = plan["n_nodes"]
    NPAD, NBP, BPC = plan["NPAD"], plan["NBP"], plan["BPC"]
    HALF = plan["HALF"]
    C_LO, C_HI, C, choff = plan["C_LO"], plan["C_HI"], plan["C"], plan["choff"]
    CH_TOT, L_TOT = plan["CH_TOT"], plan["L_TOT"]
    CMAX = int(max(C))
    KC = d_in // P          # feature chunks (4)
    assert d_in % P == 0 and d_out % P == 0

    nc = bacc.Bacc("TRN2")

    x16 = nc.declare_dram_parameter("x16", [NPAD, d_in], F16, isOutput=False)
    dego = nc.declare_dram_parameter("dego", [P, BPC], F32, isOutput=False)
    degl = nc.declare_dram_parameter("degl", [P, CH_TOT], F32, isOutput=False)
    idxp = nc.declare_dram_parameter("idx", [P, L_TOT // 16], I16, isOutput=False)
    clocp = nc.declare_dram_parameter("cloc", [P, CH_TOT], F16, isOutput=False)
    attrp = nc.declare_dram_parameter("attr", [3, P, CH_TOT], F32, isOutput=False)
    confp = nc.declare_dram_parameter("confr", [P, 4], F32, isOutput=False)
    wtp = nc.declare_dram_parameter("wt", [d_in, d_out], F16, isOutput=False)
    biasp = nc.declare_dram_parameter("biasr", [P, d_out], F32, isOutput=False)
    iotap = nc.declare_dram_parameter("iota", [P, P], F16, isOutput=False)
    identp = nc.declare_dram_parameter("ident", [P, P], F16, isOutput=False)
    yout = nc.declare_dram_parameter("y", [BPC * P, d_out], F32, isOutput=True)

    with tile.TileContext(nc) as tc:
        with (
            tc.tile_pool(name="const", bufs=1) as cp,
            tc.tile_pool(name="wide", bufs=2) as wp,
            tc.tile_pool(name="gath", bufs=2) as gp,
            tc.tile_pool(name="work", bufs=2) as wk,
            tc.tile_pool(name="psum", bufs=2, space="PSUM") as pp,
        ):
            # ---- constants ----
            iota_sb = cp.tile([P, 1, P], F16)
            nc.sync.dma_start(out=iota_sb[:, 0, :], in_=iotap[:, :])
            ident_sb = cp.tile([P, P], F16)
            nc.sync.dma_start(out=ident_sb[:], in_=identp[:, :])
            conf_sb = cp.tile([P, 4], F32)
            nc.sync.dma_start(out=conf_sb[:], in_=confp[:, :])
            bias_sb = cp.tile([P, d_out], F32)
            nc.sync.dma_start(out=bias_sb[:], in_=biasp[:, :])
            wt_sb = cp.tile([P, KC, d_out], F16)
            nc.sync.dma_start(
                out=wt_sb[:], in_=wtp[:, :].rearrange("(kc p) o -> p kc o", p=P)
            )
            cloc_sb = cp.tile([P, CH_TOT], F16)
            nc.sync.dma_start(out=cloc_sb[:], in_=clocp[:, :])
            idx_sb = cp.tile([P, L_TOT // 16], I16)
            nc.sync.dma_start(out=idx_sb[:], in_=idxp[:, :])
            att_sb = cp.tile([P, 3, CH_TOT], F32)
            nc.sync.dma_start(
                out=att_sb[:], in_=attrp[:, :, :].rearrange("t p c -> p t c")
            )

            # ---- dis = where(deg>0, 1/sqrt(deg), 0) ----
            def masked_rsqrt(dst, deg_sb, n):
                safe = cp.tile(list(deg_sb.shape), F32, tag=f"rs_safe{n}")
                nc.vector.tensor_scalar_max(safe[:], deg_sb[:], 1.0)
                nc.scalar.sqrt(safe[:], safe[:])
                nc.vector.reciprocal(safe[:], safe[:])
                mask = cp.tile(list(deg_sb.shape), F32, tag=f"rs_mask{n}")
                nc.vector.tensor_scalar(
                    mask[:], deg_sb[:], 0.0, None, mybir.AluOpType.is_gt
                )
                nc.vector.tensor_tensor(
                    dst[:], safe[:], mask[:], mybir.AluOpType.mult
                )

            dego_sb = cp.tile([P, BPC], F32)
            nc.sync.dma_start(out=dego_sb[:], in_=dego[:, :])
            diso_sb = cp.tile([P, BPC], F32)
            masked_rsqrt(diso_sb, dego_sb, "o")

            # dis[row_e] per lane, from host-supplied per-lane source degree
            degl_sb = cp.tile([P, CH_TOT], F32, tag="degl")
            nc.sync.dma_start(out=degl_sb[:], in_=degl[:, :])
            disl_sb = cp.tile([P, CH_TOT], F32, tag="disl")
            masked_rsqrt(disl_sb, degl_sb, "l")

            # ---- w2 = sigmoid(attr @ conf_w + conf_b) * dis[row] per lane ----
            acc = cp.tile([P, CH_TOT], F32, tag="wacc")
            tmp = cp.tile([P, CH_TOT], F32, tag="wtmp")
            nc.vector.tensor_scalar_mul(acc[:], att_sb[:, 0, :], conf_sb[:, 0:1])
            nc.vector.tensor_scalar_mul(tmp[:], att_sb[:, 1, :], conf_sb[:, 1:2])
            nc.vector.tensor_tensor(acc[:], acc[:], tmp[:], mybir.AluOpType.add)
            nc.vector.tensor_scalar_mul(tmp[:], att_sb[:, 2, :], conf_sb[:, 2:3])
            nc.vector.tensor_tensor(acc[:], acc[:], tmp[:], mybir.AluOpType.add)
            wsig = cp.tile([P, CH_TOT], F32, tag="wsig")
            nc.scalar.activation(
                wsig[:], acc[:], mybir.ActivationFunctionType.Sigmoid,
                bias=conf_sb[:, 3:4], scale=1.0,
            )
            w_sb = cp.tile([P, CH_TOT], F16, tag="w2")
            nc.vector.tensor_tensor(
                w_sb[:], wsig[:], disl_sb[:], mybir.AluOpType.mult
            )

            # ---- per destination-block pipeline ----
            for b in range(BPC):
                cb = int(C[b])
                clo, chi = int(C_LO[b]), int(C_HI[b])
                co = int(choff[b])
                y_sb = wk.tile([P, d_out], F32, tag="ysb")
                if cb == 0:
                    nc.vector.tensor_copy(out=y_sb[:], in_=bias_sb[:])
                    nc.sync.dma_start(
                        out=yout[b * P : (b + 1) * P, :], in_=y_sb[:]
                    )
                    continue

                xg = gp.tile([P, CMAX, d_in], F16, tag="xg")

                # HW dma_gather caps at 1024 indices (8 chunks) per
                # instruction (Q7 per-core batch limit) - split.
                MAXCH = 8

                def emit_gathers(dst, src_ap, dst0, idx0, nch, elem):
                    for s in range(0, nch, MAXCH):
                        n = min(MAXCH, nch - s)
                        nc.gpsimd.dma_gather(
                            dst[:, dst0 + s : dst0 + s + n, :], src_ap,
                            idx_sb[:, (idx0 + s) * 8 : (idx0 + s + n) * 8],
                            n * P, n * P, elem,
                        )

                if "no_gather_x" in debug_mode:
                    nc.vector.memset(xg[:, :cb, :], 0.25)
                else:
                    emit_gathers(xg, x16[:, :], 0, co, clo, d_in)
                    emit_gathers(xg, x16[HALF:, :], clo, co + clo, chi, d_in)

                if "no_pe" in debug_mode:
                    nc.vector.tensor_copy(out=y_sb[:], in_=bias_sb[:])
                    nc.sync.dma_start(
                        out=yout[b * P : (b + 1) * P, :], in_=y_sb[:]
                    )
                    continue

                # Wsel[e, c, d] = w_e * (col_local == d)
                wsel = wk.tile([P, CMAX, P], F16, tag="wsel")
                nc.vector.tensor_tensor(
                    wsel[:, :cb, :],
                    cloc_sb[:, co : co + cb, None].to_broadcast([P, cb, P]),
                    iota_sb[:, 0:1, :].to_broadcast([P, cb, P]),
                    mybir.AluOpType.is_equal,
                )
                nc.vector.tensor_tensor(
                    wsel[:, :cb, :],
                    wsel[:, :cb, :],
                    w_sb[:, co : co + cb, None].to_broadcast([P, cb, P]),
                    mybir.AluOpType.mult,
                )

                # SpMM: psum[d, k] += sum_e Wsel[e, d] * Xg[e, k]
                ps = pp.tile([P, d_in], F32, tag="spmm")
                for cc in range(cb):
                    nc.tensor.matmul(
                        ps[:], lhsT=wsel[:, cc, :], rhs=xg[:, cc, :],
                        start=(cc == 0), stop=(cc == cb - 1),
                    )
                spmm_sb = wk.tile([P, d_in], F16, tag="spmmsb")
                nc.vector.tensor_copy(out=spmm_sb[:], in_=ps[:])

                # transpose [d, k] -> [k, d] per 128-chunk
                pst = pp.tile([P, KC, P], F16, tag="tr")
                for kc in range(KC):
                    nc.tensor.transpose(
                        pst[:, kc, :], spmm_sb[:, kc * P : (kc + 1) * P], ident_sb[:]
                    )
                outT = wk.tile([P, KC, P], F16, tag="outT")
                nc.vector.tensor_copy(out=outT[:], in_=pst[:])

                # linear: y[d, o] = sum_k outT[k, d] * WT[k, o]
                py = pp.tile([P, d_out], F32, tag="ylin")
                for kc in range(KC):
                    nc.tensor.matmul(
                        py[:], lhsT=outT[:, kc, :], rhs=wt_sb[:, kc, :],
                        start=(kc == 0), stop=(kc == KC - 1),
                    )
                nc.vector.tensor_scalar_mul(y_sb[:], py[:], diso_sb[:, b : b + 1])
                nc.vector.tensor_tensor(
                    y_sb[:], y_sb[:], bias_sb[:], mybir.AluOpType.add
                )
                nc.sync.dma_start(out=yout[b * P : (b + 1) * P, :], in_=y_sb[:])

    nc.compile()
    return nc


def make_in_maps(plan, core_arrays, x, lin_w, lin_b, conf_w, conf_b, edge_index):
    n_nodes = plan["n_nodes"]
    NBP, BPC, NPAD = plan["NBP"], plan["BPC"], plan["NPAD"]
    row = np.asarray(edge_index[0], dtype=np.int64)

    x16 = np.zeros((NPAD, x.shape[1]), dtype=np.float16)
    x16[: x.shape[0]] = np.asarray(x, np.float32).astype(np.float16)
    deg = np.bincount(row, minlength=NPAD).astype(np.float32)
    degt = np.ascontiguousarray(deg.reshape(NBP, P).T)  # [128, NBP]
    CH_TOT = plan["CH_TOT"]
    conf_row = np.concatenate(
        [np.asarray(conf_w, np.float32).reshape(-1), np.asarray(conf_b, np.float32).reshape(-1)]
    ).astype(np.float32)
    confr = np.tile(conf_row, (P, 1))
    wt = np.ascontiguousarray(np.asarray(lin_w, np.float32).T.astype(np.float16))
    biasr = np.tile(np.asarray(lin_b, np.float32).reshape(1, -1), (P, 1)).astype(np.float32)
    iota = np.tile(np.arange(P, dtype=np.float16), (P, 1))
    ident = np.eye(P, dtype=np.float16)

    in_maps = []
    for c in range(plan["n_cores"]):
        arr = core_arrays[c]
        degl = np.zeros((P, CH_TOT), dtype=np.float32)
        degl[arr["lanes"] % P, arr["lanes"] // P] = deg[arr["rows"]]
        in_maps.append({
            "x16": x16,
            "degl": degl,
            "dego": np.ascontiguousarray(degt[:, c * BPC : (c + 1) * BPC]),
            "idx": arr["idx"],
            "cloc": arr["cloc"],
            "attr": arr["attr"],
            "confr": confr,
            "wt": wt,
            "biasr": biasr,
            "iota": iota,
            "ident": ident,
        })
    return in_maps


def _run(x, edge_index, edge_attr, lin_w, lin_b, conf_w, conf_b, **run_kwargs):
    n_nodes, d_in = x.shape
    d_out = lin_w.shape[0]
    plan = make_plan(edge_index, n_nodes)
    core_arrays = make_core_arrays(plan, edge_index, edge_attr)
    nc = build_program(plan, d_in, d_out)
    in_maps = make_in_maps(plan, core_arrays, x, lin_w, lin_b, conf_w, conf_b, edge_index)
    res = run_bass_kernel_spmd(nc, in_maps, list(range(plan["n_cores"])), **run_kwargs)
    ys = [np.asarray(res.results[c]["y"], np.float32) for c in range(plan["n_cores"])]
    y = np.concatenate(ys, axis=0)[:n_nodes]
    return y, res


def kernel(x, edge_index, edge_attr, lin_w, lin_b, conf_w, conf_b):
    y, _ = _run(x, edge_index, edge_attr, lin_w, lin_b, conf_w, conf_b)
    return y.astype(np.float32)


# revision 27
# speedup vs baseline: 2.0287x; 1.1151x over previous
"""Trainium2 Bass kernel for ConfidenceGCNConv message passing.

Math (reference):
    w_e   = sigmoid(edge_attr @ conf_w.T + conf_b)            # [E]
    deg   = bincount(row); dis = where(deg>0, rsqrt(deg), 0)  # [N]
    out[c] = sum_{e: col_e=c} dis[row_e]*dis[col_e]*w_e * x[row_e]
    y     = out @ lin_w.T + lin_b

Device strategy (8 cores, destination-partitioned):
  - Host: partition edges by destination block (128 dests / block, 49 blocks
    per core), split per block into row<HALF / row>=HALF halves (int16 gather
    index limit), pad each half to whole 128-lane chunks. Pure index work.
  - Device per core, per destination block b:
      * dma_gather x16[row_e] rows (fp16, 1KB each) -> Xg [128e, C, 512]
      * dma_gather dis_wide[row_e] (256B rows, col 0 = dis[row]) -> Dr
      * Wsel[e, d] = w_e * dis[row_e] * (col_local_e == d)   (DVE iota-compare)
      * PSUM spmm[128d, 512k] += sum_c Wsel_c^T @ Xg_c       (PE, contraction=edges)
      * transpose spmm -> outT [128k, 4, 128d]               (PE identity trick)
      * PSUM y[128d, 512o] = sum_kc outT_kc^T @ WT_kc        (PE)
      * y_sb = y * dis[dest] + bias                          (scale commutes with @W)
  - w_e = sigmoid(...) and dis = masked 1/sqrt(deg) computed on device.
    dis_wide (replicated 128-wide dis table for 256B-aligned gathers) is
    built on device once per core.
"""

import sys

for _p in ("/opt/trn_rl_repo",):
    if _p not in sys.path:
        sys.path.insert(0, _p)

import numpy as np

import concourse.bass as bass
import concourse.mybir as mybir
import concourse.tile as tile
from concourse import bacc
from concourse.bass_utils import run_bass_kernel_spmd

P = 128
NCORES = 8
F16 = mybir.dt.float16
F32 = mybir.dt.float32
I16 = mybir.dt.int16


def _cdiv(a, b):
    return (a + b - 1) // b


def make_plan(edge_index, n_nodes, n_cores=NCORES):
    """Host-side integer/index preprocessing: edge partition + padded layout.

    Returns a dict with static program metadata (shared across cores) and the
    per-edge placement (core, lane) used to build per-core input arrays.
    """
    row = np.asarray(edge_index[0], dtype=np.int64)
    col = np.asarray(edge_index[1], dtype=np.int64)
    E = row.shape[0]

    NB = _cdiv(n_nodes, P)           # blocks covering all nodes
    BPC = _cdiv(NB, n_cores)         # blocks per core
    NBP = BPC * n_cores              # padded total blocks
    NPAD = NBP * P                   # padded node count (dest side)
    # int16 gather indices need both halves < 32768. Within that window,
    # bias the split so the lo side of most blocks fills exactly 8 chunks
    # (one dma_gather): fewer descriptors + fewer per-instruction overheads.
    HALF = min(n_nodes // 2, max(n_nodes - 32768, 23424))
    assert HALF <= 32768 and (n_nodes - HALF) <= 32768

    gb = col // P                    # global destination block
    core = gb // BPC
    b = gb - core * BPC              # block local to core
    half = (row >= HALF).astype(np.int64)
    key = (core * BPC + b) * 2 + half

    order = np.argsort(key, kind="stable")
    counts = np.bincount(key, minlength=2 * NBP)
    cnt3 = counts.reshape(n_cores, BPC, 2)
    chunks_needed = _cdiv(cnt3, P)
    C_LO = chunks_needed[:, :, 0].max(axis=0)  # [BPC], shared across cores
    C_HI = chunks_needed[:, :, 1].max(axis=0)
    C = C_LO + C_HI
    choff = np.zeros(BPC + 1, dtype=np.int64)
    np.cumsum(C, out=choff[1:])
    CH_TOT = int(choff[-1])
    L_TOT = CH_TOT * P

    # lane base per key (core-independent): lo at choff[b]*128, hi after lo
    lane_lo = choff[:BPC] * P
    lane_hi = lane_lo + C_LO * P
    lane_base_k = np.zeros(2 * NBP, dtype=np.int64)
    for c in range(n_cores):
        lane_base_k[(c * BPC + np.arange(BPC)) * 2 + 0] = lane_lo
        lane_base_k[(c * BPC + np.arange(BPC)) * 2 + 1] = lane_hi

    sorted_key = key[order]
    starts = np.zeros(2 * NBP, dtype=np.int64)
    np.cumsum(counts[:-1], out=starts[1:])
    pos_in_grp = np.arange(E, dtype=np.int64) - starts[sorted_key]
    lane = lane_base_k[sorted_key] + pos_in_grp
    edge_core = sorted_key // (2 * BPC)

    return dict(
        n_nodes=n_nodes, E=E, NB=NB, BPC=BPC, NBP=NBP, NPAD=NPAD, HALF=HALF,
        C_LO=C_LO.astype(int), C_HI=C_HI.astype(int), C=C.astype(int),
        choff=choff.astype(int), CH_TOT=CH_TOT, L_TOT=L_TOT,
        order=order, lane=lane, edge_core=edge_core,
        n_cores=n_cores,
    )


def make_core_arrays(plan, edge_index, edge_attr):
    """Per-core padded gather-index / col-local / edge-attr arrays."""
    row = np.asarray(edge_index[0], dtype=np.int64)
    col = np.asarray(edge_index[1], dtype=np.int64)
    HALF = plan["HALF"]
    CH_TOT = plan["CH_TOT"]
    L_TOT = plan["L_TOT"]
    out = []
    for c in range(plan["n_cores"]):
        sel = plan["edge_core"] == c
        e_sel = plan["order"][sel]
        lanes = plan["lane"][sel]
        rows = row[e_sel]
        idxval = np.where(rows < HALF, rows, rows - HALF).astype(np.int16)

        idx2d = np.zeros((16, L_TOT // 16), dtype=np.int16)
        idx2d[lanes % 16, lanes // 16] = idxval
        # HW reads the 16-partition wrap replicated across all 8 Q7 core
        # groups (sim only reads partitions 0-15; HW cores read their own).
        idx2d = np.tile(idx2d, (8, 1))

        cloc = np.full((P, CH_TOT), float(P), dtype=np.float16)  # 128 = sentinel
        cloc[lanes % P, lanes // P] = (col[e_sel] % P).astype(np.float16)

        attr3 = np.zeros((3, P, CH_TOT), dtype=np.float32)
        attr3[:, lanes % P, lanes // P] = np.asarray(edge_attr, np.float32)[e_sel].T

        out.append(dict(idx=idx2d, cloc=cloc, attr=attr3, rows=rows, lanes=lanes))
    return out


def build_program(plan, d_in, d_out, debug_mode=()):
    """Build the (SPMD-shared) Bass program.

    debug_mode: set of strings to disable pieces for HW bisection:
      'no_gather_x'  - memset xg instead of dma_gather from x16
      'no_gather_d'  - memset dr instead of dma_gather from dis_wide
      'no_diswide'   - skip dis_wide build
      'no_pe'        - skip matmul/transpose; y = bias only
    """
    debug_mode = set(debug_mode)
    n_nodes = plan["n_nodes"]
    NPAD, NBP, BPC = plan["NPAD"], plan["NBP"], plan["BPC"]
    HALF = plan["HALF"]
    C_LO, C_HI, C, choff = plan["C_LO"], plan["C_HI"], plan["C"], plan["choff"]
    CH_TOT, L_TOT = plan["CH_TOT"], plan["L_TOT"]
    CMAX = int(max(C))
    KC = d_in // P          # feature chunks (4)
    assert d_in % P == 0 and d_out % P == 0

    nc = bacc.Bacc("TRN2")

    x16 = nc.declare_dram_parameter("x16", [NPAD, d_in], F16, isOutput=False)
    dego = nc.declare_dram_parameter("dego", [P, BPC], F32, isOutput=False)
    degl = nc.declare_dram_parameter("degl", [P, CH_TOT], F32, isOutput=False)
    idxp = nc.declare_dram_parameter("idx", [P, L_TOT // 16], I16, isOutput=False)
    clocp = nc.declare_dram_parameter("cloc", [P, CH_TOT], F16, isOutput=False)
    attrp = nc.declare_dram_parameter("attr", [3, P, CH_TOT], F32, isOutput=False)
    confp = nc.declare_dram_parameter("confr", [P, 4], F32, isOutput=False)
    wtp = nc.declare_dram_parameter("wt", [d_in, d_out], F16, isOutput=False)
    biasp = nc.declare_dram_parameter("biasr", [P, d_out], F32, isOutput=False)
    iotap = nc.declare_dram_parameter("iota", [P, P], F16, isOutput=False)
    identp = nc.declare_dram_parameter("ident", [P, P], F16, isOutput=False)
    yout = nc.declare_dram_parameter("y", [BPC * P, d_out], F32, isOutput=True)

    with tile.TileContext(nc) as tc:
        with (
            tc.tile_pool(name="const", bufs=1) as cp,
            tc.tile_pool(name="wide", bufs=2) as wp,
            tc.tile_pool(name="gath", bufs=2) as gp,
            tc.tile_pool(name="work", bufs=2) as wk,
            tc.tile_pool(name="psum", bufs=2, space="PSUM") as pp,
        ):
            # ---- constants ----
            iota_sb = cp.tile([P, 1, P], F16)
            nc.sync.dma_start(out=iota_sb[:, 0, :], in_=iotap[:, :])
            ident_sb = cp.tile([P, P], F16)
            nc.sync.dma_start(out=ident_sb[:], in_=identp[:, :])
            conf_sb = cp.tile([P, 4], F32)
            nc.sync.dma_start(out=conf_sb[:], in_=confp[:, :])
            bias_sb = cp.tile([P, d_out], F32)
            nc.sync.dma_start(out=bias_sb[:], in_=biasp[:, :])
            wt_sb = cp.tile([P, KC, d_out], F16)
            nc.sync.dma_start(
                out=wt_sb[:], in_=wtp[:, :].rearrange("(kc p) o -> p kc o", p=P)
            )
            cloc_sb = cp.tile([P, CH_TOT], F16)
            nc.sync.dma_start(out=cloc_sb[:], in_=clocp[:, :])
            idx_sb = cp.tile([P, L_TOT // 16], I16)
            nc.sync.dma_start(out=idx_sb[:], in_=idxp[:, :])
            att_sb = cp.tile([P, 3, CH_TOT], F32)
            nc.sync.dma_start(
                out=att_sb[:], in_=attrp[:, :, :].rearrange("t p c -> p t c")
            )

            # ---- dis = where(deg>0, 1/sqrt(deg), 0) ----
            def masked_rsqrt(dst, deg_sb, n):
                safe = cp.tile(list(deg_sb.shape), F32, tag=f"rs_safe{n}")
                nc.vector.tensor_scalar_max(safe[:], deg_sb[:], 1.0)
                nc.scalar.sqrt(safe[:], safe[:])
                nc.vector.reciprocal(safe[:], safe[:])
                mask = cp.tile(list(deg_sb.shape), F32, tag=f"rs_mask{n}")
                nc.vector.tensor_scalar(
                    mask[:], deg_sb[:], 0.0, None, mybir.AluOpType.is_gt
                )
                nc.vector.tensor_tensor(
                    dst[:], safe[:], mask[:], mybir.AluOpType.mult
                )

            dego_sb = cp.tile([P, BPC], F32)
            nc.sync.dma_start(out=dego_sb[:], in_=dego[:, :])
            diso_sb = cp.tile([P, BPC], F32)
            masked_rsqrt(diso_sb, dego_sb, "o")

            # dis[row_e] per lane, from host-supplied per-lane source degree
            degl_sb = cp.tile([P, CH_TOT], F32, tag="degl")
            nc.sync.dma_start(out=degl_sb[:], in_=degl[:, :])
            disl_sb = cp.tile([P, CH_TOT], F32, tag="disl")
            masked_rsqrt(disl_sb, degl_sb, "l")

            # ---- w2 = sigmoid(attr @ conf_w + conf_b) * dis[row] per lane ----
            acc = cp.tile([P, CH_TOT], F32, tag="wacc")
            tmp = cp.tile([P, CH_TOT], F32, tag="wtmp")
            nc.vector.tensor_scalar_mul(acc[:], att_sb[:, 0, :], conf_sb[:, 0:1])
            nc.vector.tensor_scalar_mul(tmp[:], att_sb[:, 1, :], conf_sb[:, 1:2])
            nc.vector.tensor_tensor(acc[:], acc[:], tmp[:], mybir.AluOpType.add)
            nc.vector.tensor_scalar_mul(tmp[:], att_sb[:, 2, :], conf_sb[:, 2:3])
            nc.vector.tensor_tensor(acc[:], acc[:], tmp[:], mybir.AluOpType.add)
            wsig = cp.tile([P, CH_TOT], F32, tag="wsig")
            nc.scalar.activation(
                wsig[:], acc[:], mybir.ActivationFunctionType.Sigmoid,
                bias=conf_sb[:, 3:4], scale=1.0,
            )
            w_sb = cp.tile([P, CH_TOT], F16, tag="w2")
            nc.vector.tensor_tensor(
                w_sb[:], wsig[:], disl_sb[:], mybir.AluOpType.mult
            )

            # ---- per destination-block pipeline ----
            for b in range(BPC):
                cb = int(C[b])
                clo, chi = int(C_LO[b]), int(C_HI[b])
                co = int(choff[b])
                y_sb = wk.tile([P, d_out], F32, tag="ysb")
                if cb == 0:
                    nc.vector.tensor_copy(out=y_sb[:], in_=bias_sb[:])
                    nc.sync.dma_start(
                        out=yout[b * P : (b + 1) * P, :], in_=y_sb[:]
                    )
                    continue

                xg = gp.tile([P, CMAX, d_in], F16, tag="xg")

                # HW dma_gather caps at 1024 indices (8 chunks) per
                # instruction (Q7 per-core batch limit) - split.
                MAXCH = 8

                def emit_gathers(dst, src_ap, dst0, idx0, nch, elem):
                    for s in range(0, nch, MAXCH):
                        n = min(MAXCH, nch - s)
                        nc.gpsimd.dma_gather(
                            dst[:, dst0 + s : dst0 + s + n, :], src_ap,
                            idx_sb[:, (idx0 + s) * 8 : (idx0 + s + n) * 8],
                            n * P, n * P, elem,
                        )

                if "no_gather_x" in debug_mode:
                    nc.vector.memset(xg[:, :cb, :], 0.25)
                else:
                    emit_gathers(xg, x16[:, :], 0, co, clo, d_in)
                    emit_gathers(xg, x16[HALF:, :], clo, co + clo, chi, d_in)

                if "no_pe" in debug_mode:
                    nc.vector.tensor_copy(out=y_sb[:], in_=bias_sb[:])
                    nc.sync.dma_start(
                        out=yout[b * P : (b + 1) * P, :], in_=y_sb[:]
                    )
                    continue

                # Wsel[e, c, d] = w_e * (col_local == d)
                wsel = wk.tile([P, CMAX, P], F16, tag="wsel")
                nc.vector.tensor_tensor(
                    wsel[:, :cb, :],
                    cloc_sb[:, co : co + cb, None].to_broadcast([P, cb, P]),
                    iota_sb[:, 0:1, :].to_broadcast([P, cb, P]),
                    mybir.AluOpType.is_equal,
                )
                nc.vector.tensor_tensor(
                    wsel[:, :cb, :],
                    wsel[:, :cb, :],
                    w_sb[:, co : co + cb, None].to_broadcast([P, cb, P]),
                    mybir.AluOpType.mult,
                )

                # SpMM: psum[d, k] += sum_e Wsel[e, d] * Xg[e, k]
                ps = pp.tile([P, d_in], F32, tag="spmm")
                for cc in range(cb):
                    nc.tensor.matmul(
                        ps[:], lhsT=wsel[:, cc, :], rhs=xg[:, cc, :],
                        start=(cc == 0), stop=(cc == cb - 1),
                    )
                spmm_sb = wk.tile([P, d_in], F16, tag="spmmsb")
                nc.vector.tensor_copy(out=spmm_sb[:], in_=ps[:])

                # transpose [d, k] -> [k, d] per 128-chunk
                pst = pp.tile([P, KC, P], F16, tag="tr")
                for kc in range(KC):
                    nc.tensor.transpose(
                        pst[:, kc, :], spmm_sb[:, kc * P : (kc + 1) * P], ident_sb[:]
                    )
                outT = wk.tile([P, KC, P], F16, tag="outT")
                nc.vector.tensor_copy(out=outT[:], in_=pst[:])

                # linear: y[d, o] = sum_k outT[k, d] * WT[k, o]
                py = pp.tile([P, d_out], F32, tag="ylin")
                for kc in range(KC):
                    nc.tensor.matmul(
                        py[:], lhsT=outT[:, kc, :], rhs=wt_sb[:, kc, :],
                        start=(kc == 0), stop=(kc == KC - 1),
                    )
                nc.vector.tensor_scalar_mul(y_sb[:], py[:], diso_sb[:, b : b + 1])
                nc.vector.tensor_tensor(
                    y_sb[:], y_sb[:], bias_sb[:], mybir.AluOpType.add
                )
                nc.sync.dma_start(out=yout[b * P : (b + 1) * P, :], in_=y_sb[:])

    nc.compile()
    return nc


def make_in_maps(plan, core_arrays, x, lin_w, lin_b, conf_w, conf_b, edge_index):
    n_nodes = plan["n_nodes"]
    NBP, BPC, NPAD = plan["NBP"], plan["BPC"], plan["NPAD"]
    row = np.asarray(edge_index[0], dtype=np.int64)

    x16 = np.zeros((NPAD, x.shape[1]), dtype=np.float16)
    x16[: x.shape[0]] = np.asarray(x, np.float32).astype(np.float16)
    deg = np.bincount(row, minlength=NPAD).astype(np.float32)
    degt = np.ascontiguousarray(deg.reshape(NBP, P).T)  # [128, NBP]
    CH_TOT = plan["CH_TOT"]
    conf_row = np.concatenate(
        [np.asarray(conf_w, np.float32).reshape(-1), np.asarray(conf_b, np.float32).reshape(-1)]
    ).astype(np.float32)
    confr = np.tile(conf_row, (P, 1))
    wt = np.ascontiguousarray(np.asarray(lin_w, np.float32).T.astype(np.float16))
    biasr = np.tile(np.asarray(lin_b, np.float32).reshape(1, -1), (P, 1)).astype(np.float32)
    iota = np.tile(np.arange(P, dtype=np.float16), (P, 1))
    ident = np.eye(P, dtype=np.float16)

    in_maps = []
    for c in range(plan["n_cores"]):
        arr = core_arrays[c]
        degl = np.zeros((P, CH_TOT), dtype=np.float32)
        degl[arr["lanes"] % P, arr["lanes"] // P] = deg[arr["rows"]]
        in_maps.append({
            "x16": x16,
            "degl": degl,
            "dego": np.ascontiguousarray(degt[:, c * BPC : (c + 1) * BPC]),
            "idx": arr["idx"],
            "cloc": arr["cloc"],
            "attr": arr["attr"],
            "confr": confr,
            "wt": wt,
            "biasr": biasr,
            "iota": iota,
            "ident": ident,
        })
    return in_maps


def _run(x, edge_index, edge_attr, lin_w, lin_b, conf_w, conf_b, **run_kwargs):
    n_nodes, d_in = x.shape
    d_out = lin_w.shape[0]
    plan = make_plan(edge_index, n_nodes)
    core_arrays = make_core_arrays(plan, edge_index, edge_attr)
    nc = build_program(plan, d_in, d_out)
    in_maps = make_in_maps(plan, core_arrays, x, lin_w, lin_b, conf_w, conf_b, edge_index)
    res = run_bass_kernel_spmd(nc, in_maps, list(range(plan["n_cores"])), **run_kwargs)
    ys = [np.asarray(res.results[c]["y"], np.float32) for c in range(plan["n_cores"])]
    y = np.concatenate(ys, axis=0)[:n_nodes]
    return y, res


def kernel(x, edge_index, edge_attr, lin_w, lin_b, conf_w, conf_b):
    y, _ = _run(x, edge_index, edge_attr, lin_w, lin_b, conf_w, conf_b)
    return y.astype(np.float32)


# revision 32
# speedup vs baseline: 4.1747x; 2.0578x over previous
"""Trainium2 Bass kernel for ConfidenceGCNConv message passing.

Math (reference):
    w_e   = sigmoid(edge_attr @ conf_w.T + conf_b)            # [E]
    deg   = bincount(row); dis = where(deg>0, rsqrt(deg), 0)  # [N]
    out[c] = sum_{e: col_e=c} dis[row_e]*dis[col_e]*w_e * x[row_e]
    y     = out @ lin_w.T + lin_b

Device strategy (8 cores, destination-partitioned):
  - Host: partition edges by destination block (128 dests / block, 49 blocks
    per core), split per block into row<HALF / row>=HALF halves (int16 gather
    index limit), pad each half to whole 128-lane chunks. Pure index work.
  - Device per core, per destination block b:
      * dma_gather x16[row_e] rows (fp16, 1KB each) -> Xg [128e, C, 512]
      * dma_gather dis_wide[row_e] (256B rows, col 0 = dis[row]) -> Dr
      * Wsel[e, d] = w_e * dis[row_e] * (col_local_e == d)   (DVE iota-compare)
      * PSUM spmm[128d, 512k] += sum_c Wsel_c^T @ Xg_c       (PE, contraction=edges)
      * transpose spmm -> outT [128k, 4, 128d]               (PE identity trick)
      * PSUM y[128d, 512o] = sum_kc outT_kc^T @ WT_kc        (PE)
      * y_sb = y * dis[dest] + bias                          (scale commutes with @W)
  - w_e = sigmoid(...) and dis = masked 1/sqrt(deg) computed on device.
    dis_wide (replicated 128-wide dis table for 256B-aligned gathers) is
    built on device once per core.
"""

import sys

for _p in ("/opt/trn_rl_repo",):
    if _p not in sys.path:
        sys.path.insert(0, _p)

import numpy as np

import concourse.bass as bass
import concourse.mybir as mybir
import concourse.tile as tile
from concourse import bacc
from concourse.bass_utils import run_bass_kernel_spmd

P = 128
NCORES = 8
F16 = mybir.dt.float16
F32 = mybir.dt.float32
I16 = mybir.dt.int16


def _cdiv(a, b):
    return (a + b - 1) // b


def make_plan(edge_index, n_nodes, n_cores=NCORES):
    """Host-side integer/index preprocessing: edge partition + padded layout.

    Returns a dict with static program metadata (shared across cores) and the
    per-edge placement (core, lane) used to build per-core input arrays.
    """
    row = np.asarray(edge_index[0], dtype=np.int64)
    col = np.asarray(edge_index[1], dtype=np.int64)
    E = row.shape[0]

    NB = _cdiv(n_nodes, P)           # blocks covering all nodes
    BPC = _cdiv(NB, n_cores)         # blocks per core
    NBP = BPC * n_cores              # padded total blocks
    NPAD = NBP * P                   # padded node count (dest side)
    # int16 gather indices need both halves < 32768. Within that window,
    # bias the split so the lo side of most blocks fills exactly 8 chunks
    # (one dma_gather): fewer descriptors + fewer per-instruction overheads.
    HALF = min(n_nodes // 2, max(n_nodes - 32768, 23424))
    assert HALF <= 32768 and (n_nodes - HALF) <= 32768

    gb = col // P                    # global destination block
    core = gb // BPC
    b = gb - core * BPC              # block local to core
    half = (row >= HALF).astype(np.int64)
    key = (core * BPC + b) * 2 + half

    order = np.argsort(key, kind="stable")
    counts = np.bincount(key, minlength=2 * NBP)
    cnt3 = counts.reshape(n_cores, BPC, 2)
    chunks_needed = _cdiv(cnt3, P)
    C_LO = chunks_needed[:, :, 0].max(axis=0)  # [BPC], shared across cores
    C_HI = chunks_needed[:, :, 1].max(axis=0)
    C = C_LO + C_HI
    choff = np.zeros(BPC + 1, dtype=np.int64)
    np.cumsum(C, out=choff[1:])
    CH_TOT = int(choff[-1])
    L_TOT = CH_TOT * P

    # lane base per key (core-independent): lo at choff[b]*128, hi after lo
    lane_lo = choff[:BPC] * P
    lane_hi = lane_lo + C_LO * P
    lane_base_k = np.zeros(2 * NBP, dtype=np.int64)
    for c in range(n_cores):
        lane_base_k[(c * BPC + np.arange(BPC)) * 2 + 0] = lane_lo
        lane_base_k[(c * BPC + np.arange(BPC)) * 2 + 1] = lane_hi

    sorted_key = key[order]
    starts = np.zeros(2 * NBP, dtype=np.int64)
    np.cumsum(counts[:-1], out=starts[1:])
    pos_in_grp = np.arange(E, dtype=np.int64) - starts[sorted_key]
    lane = lane_base_k[sorted_key] + pos_in_grp
    edge_core = sorted_key // (2 * BPC)

    return dict(
        n_nodes=n_nodes, E=E, NB=NB, BPC=BPC, NBP=NBP, NPAD=NPAD, HALF=HALF,
        C_LO=C_LO.astype(int), C_HI=C_HI.astype(int), C=C.astype(int),
        choff=choff.astype(int), CH_TOT=CH_TOT, L_TOT=L_TOT,
        order=order, lane=lane, edge_core=edge_core,
        n_cores=n_cores,
    )


def make_core_arrays(plan, edge_index, edge_attr):
    """Per-core padded gather-index / col-local / edge-attr arrays."""
    row = np.asarray(edge_index[0], dtype=np.int64)
    col = np.asarray(edge_index[1], dtype=np.int64)
    HALF = plan["HALF"]
    CH_TOT = plan["CH_TOT"]
    L_TOT = plan["L_TOT"]
    out = []
    for c in range(plan["n_cores"]):
        sel = plan["edge_core"] == c
        e_sel = plan["order"][sel]
        lanes = plan["lane"][sel]
        rows = row[e_sel]
        idxval = np.where(rows < HALF, rows, rows - HALF).astype(np.int16)

        idx2d = np.zeros((16, L_TOT // 16), dtype=np.int16)
        idx2d[lanes % 16, lanes // 16] = idxval
        # HW reads the 16-partition wrap replicated across all 8 Q7 core
        # groups (sim only reads partitions 0-15; HW cores read their own).
        idx2d = np.tile(idx2d, (8, 1))

        cloc = np.full((P, CH_TOT), float(P), dtype=np.float16)  # 128 = sentinel
        cloc[lanes % P, lanes // P] = (col[e_sel] % P).astype(np.float16)

        attr3 = np.zeros((3, P, CH_TOT), dtype=np.float32)
        attr3[:, lanes % P, lanes // P] = np.asarray(edge_attr, np.float32)[e_sel].T

        out.append(dict(idx=idx2d, cloc=cloc, attr=attr3, rows=rows, lanes=lanes))
    return out


def build_program(plan, d_in, d_out, debug_mode=()):
    """Build the (SPMD-shared) Bass program.

    debug_mode: set of strings to disable pieces for HW bisection:
      'no_gather_x'  - memset xg instead of dma_gather from x16
      'no_gather_d'  - memset dr instead of dma_gather from dis_wide
      'no_diswide'   - skip dis_wide build
      'no_pe'        - skip matmul/transpose; y = bias only
    """
    debug_mode = set(debug_mode)
    import os as _os
    knob_sp = _os.environ.get("KNOB_SINGLE_PACKET", "1") == "1"
    knob_q = int(_os.environ.get("KNOB_QUEUES", "1"))
    n_nodes = plan["n_nodes"]
    NPAD, NBP, BPC = plan["NPAD"], plan["NBP"], plan["BPC"]
    HALF = plan["HALF"]
    C_LO, C_HI, C, choff = plan["C_LO"], plan["C_HI"], plan["C"], plan["choff"]
    CH_TOT, L_TOT = plan["CH_TOT"], plan["L_TOT"]
    CMAX = int(max(C))
    KC = d_in // P          # feature chunks (4)
    assert d_in % P == 0 and d_out % P == 0

    nc = bacc.Bacc("TRN2", num_swdge_queues=knob_q)

    x16 = nc.declare_dram_parameter("x16", [NPAD, d_in], F16, isOutput=False)
    dego = nc.declare_dram_parameter("dego", [P, BPC], F32, isOutput=False)
    degl = nc.declare_dram_parameter("degl", [P, CH_TOT], F32, isOutput=False)
    idxp = nc.declare_dram_parameter("idx", [P, L_TOT // 16], I16, isOutput=False)
    clocp = nc.declare_dram_parameter("cloc", [P, CH_TOT], F16, isOutput=False)
    attrp = nc.declare_dram_parameter("attr", [3, P, CH_TOT], F32, isOutput=False)
    confp = nc.declare_dram_parameter("confr", [P, 4], F32, isOutput=False)
    wtp = nc.declare_dram_parameter("wt", [d_in, d_out], F16, isOutput=False)
    biasp = nc.declare_dram_parameter("biasr", [P, d_out], F32, isOutput=False)
    iotap = nc.declare_dram_parameter("iota", [P, P], F16, isOutput=False)
    identp = nc.declare_dram_parameter("ident", [P, P], F16, isOutput=False)
    yout = nc.declare_dram_parameter("y", [BPC * P, d_out], F32, isOutput=True)

    with tile.TileContext(nc) as tc:
        with (
            tc.tile_pool(name="const", bufs=1) as cp,
            tc.tile_pool(name="wide", bufs=2) as wp,
            tc.tile_pool(name="gath", bufs=3) as gp,
            tc.tile_pool(name="work", bufs=3) as wk,
            tc.tile_pool(name="psum", bufs=2, space="PSUM") as pp,
        ):
            # ---- constants ----
            iota_sb = cp.tile([P, 1, P], F16)
            nc.sync.dma_start(out=iota_sb[:, 0, :], in_=iotap[:, :])
            ident_sb = cp.tile([P, P], F16)
            nc.sync.dma_start(out=ident_sb[:], in_=identp[:, :])
            conf_sb = cp.tile([P, 4], F32)
            nc.sync.dma_start(out=conf_sb[:], in_=confp[:, :])
            bias_sb = cp.tile([P, d_out], F32)
            nc.sync.dma_start(out=bias_sb[:], in_=biasp[:, :])
            wt_sb = cp.tile([P, KC, d_out], F16)
            nc.sync.dma_start(
                out=wt_sb[:], in_=wtp[:, :].rearrange("(kc p) o -> p kc o", p=P)
            )
            cloc_sb = cp.tile([P, CH_TOT], F16)
            nc.sync.dma_start(out=cloc_sb[:], in_=clocp[:, :])
            idx_sb = cp.tile([P, L_TOT // 16], I16)
            nc.sync.dma_start(out=idx_sb[:], in_=idxp[:, :])
            att_sb = cp.tile([P, 3, CH_TOT], F32)
            nc.sync.dma_start(
                out=att_sb[:], in_=attrp[:, :, :].rearrange("t p c -> p t c")
            )

            # ---- dis = where(deg>0, 1/sqrt(deg), 0) ----
            def masked_rsqrt(dst, deg_sb, n):
                safe = cp.tile(list(deg_sb.shape), F32, tag=f"rs_safe{n}")
                nc.vector.tensor_scalar_max(safe[:], deg_sb[:], 1.0)
                nc.scalar.sqrt(safe[:], safe[:])
                nc.vector.reciprocal(safe[:], safe[:])
                mask = cp.tile(list(deg_sb.shape), F32, tag=f"rs_mask{n}")
                nc.vector.tensor_scalar(
                    mask[:], deg_sb[:], 0.0, None, mybir.AluOpType.is_gt
                )
                nc.vector.tensor_tensor(
                    dst[:], safe[:], mask[:], mybir.AluOpType.mult
                )

            dego_sb = cp.tile([P, BPC], F32)
            nc.sync.dma_start(out=dego_sb[:], in_=dego[:, :])
            diso_sb = cp.tile([P, BPC], F32)
            masked_rsqrt(diso_sb, dego_sb, "o")

            # dis[row_e] per lane, from host-supplied per-lane source degree
            degl_sb = cp.tile([P, CH_TOT], F32, tag="degl")
            nc.sync.dma_start(out=degl_sb[:], in_=degl[:, :])
            disl_sb = cp.tile([P, CH_TOT], F32, tag="disl")
            masked_rsqrt(disl_sb, degl_sb, "l")

            # ---- w2 = sigmoid(attr @ conf_w + conf_b) * dis[row] per lane ----
            acc = cp.tile([P, CH_TOT], F32, tag="wacc")
            tmp = cp.tile([P, CH_TOT], F32, tag="wtmp")
            nc.vector.tensor_scalar_mul(acc[:], att_sb[:, 0, :], conf_sb[:, 0:1])
            nc.vector.tensor_scalar_mul(tmp[:], att_sb[:, 1, :], conf_sb[:, 1:2])
            nc.vector.tensor_tensor(acc[:], acc[:], tmp[:], mybir.AluOpType.add)
            nc.vector.tensor_scalar_mul(tmp[:], att_sb[:, 2, :], conf_sb[:, 2:3])
            nc.vector.tensor_tensor(acc[:], acc[:], tmp[:], mybir.AluOpType.add)
            wsig = cp.tile([P, CH_TOT], F32, tag="wsig")
            nc.scalar.activation(
                wsig[:], acc[:], mybir.ActivationFunctionType.Sigmoid,
                bias=conf_sb[:, 3:4], scale=1.0,
            )
            w_sb = cp.tile([P, CH_TOT], F16, tag="w2")
            nc.vector.tensor_tensor(
                w_sb[:], wsig[:], disl_sb[:], mybir.AluOpType.mult
            )

            # ---- per destination-block pipeline ----
            for b in range(BPC):
                cb = int(C[b])
                clo, chi = int(C_LO[b]), int(C_HI[b])
                co = int(choff[b])
                y_sb = wk.tile([P, d_out], F32, tag="ysb")
                if cb == 0:
                    nc.vector.tensor_copy(out=y_sb[:], in_=bias_sb[:])
                    nc.sync.dma_start(
                        out=yout[b * P : (b + 1) * P, :], in_=y_sb[:]
                    )
                    continue

                xg = gp.tile([P, CMAX, d_in], F16, tag="xg")

                # HW dma_gather caps at 1024 indices (8 chunks) per
                # instruction (Q7 per-core batch limit) - split.
                MAXCH = 8

                gq = [b * 4]

                def emit_gathers(dst, src_ap, dst0, idx0, nch, elem):
                    for s in range(0, nch, MAXCH):
                        n = min(MAXCH, nch - s)
                        nc.gpsimd.dma_gather(
                            dst[:, dst0 + s : dst0 + s + n, :], src_ap,
                            idx_sb[:, (idx0 + s) * 8 : (idx0 + s + n) * 8],
                            n * P, n * P, elem,
                            single_packet=knob_sp,
                            queue_num=gq[0] % knob_q,
                        )
                        gq[0] += 1

                if "no_gather_x" in debug_mode:
                    nc.vector.memset(xg[:, :cb, :], 0.25)
                else:
                    emit_gathers(xg, x16[:, :], 0, co, clo, d_in)
                    emit_gathers(xg, x16[HALF:, :], clo, co + clo, chi, d_in)

                if "no_pe" in debug_mode:
                    nc.vector.tensor_copy(out=y_sb[:], in_=bias_sb[:])
                    nc.sync.dma_start(
                        out=yout[b * P : (b + 1) * P, :], in_=y_sb[:]
                    )
                    continue

                # Wsel[e, c, d] = w_e * (col_local == d)
                wsel = wk.tile([P, CMAX, P], F16, tag="wsel")
                nc.vector.tensor_tensor(
                    wsel[:, :cb, :],
                    cloc_sb[:, co : co + cb, None].to_broadcast([P, cb, P]),
                    iota_sb[:, 0:1, :].to_broadcast([P, cb, P]),
                    mybir.AluOpType.is_equal,
                )
                nc.vector.tensor_tensor(
                    wsel[:, :cb, :],
                    wsel[:, :cb, :],
                    w_sb[:, co : co + cb, None].to_broadcast([P, cb, P]),
                    mybir.AluOpType.mult,
                )

                # SpMM: psum[d, k] += sum_e Wsel[e, d] * Xg[e, k]
                ps = pp.tile([P, d_in], F32, tag="spmm")
                for cc in range(cb):
                    nc.tensor.matmul(
                        ps[:], lhsT=wsel[:, cc, :], rhs=xg[:, cc, :],
                        start=(cc == 0), stop=(cc == cb - 1),
                    )
                spmm_sb = wk.tile([P, d_in], F16, tag="spmmsb")
                nc.scalar.copy(out=spmm_sb[:], in_=ps[:])

                # transpose [d, k] -> [k, d] per 128-chunk
                pst = pp.tile([P, KC, P], F16, tag="tr")
                for kc in range(KC):
                    nc.tensor.transpose(
                        pst[:, kc, :], spmm_sb[:, kc * P : (kc + 1) * P], ident_sb[:]
                    )
                outT = wk.tile([P, KC, P], F16, tag="outT")
                nc.scalar.copy(out=outT[:], in_=pst[:])

                # linear: y[d, o] = sum_k outT[k, d] * WT[k, o]
                py = pp.tile([P, d_out], F32, tag="ylin")
                for kc in range(KC):
                    nc.tensor.matmul(
                        py[:], lhsT=outT[:, kc, :], rhs=wt_sb[:, kc, :],
                        start=(kc == 0), stop=(kc == KC - 1),
                    )
                nc.vector.tensor_scalar_mul(y_sb[:], py[:], diso_sb[:, b : b + 1])
                nc.vector.tensor_tensor(
                    y_sb[:], y_sb[:], bias_sb[:], mybir.AluOpType.add
                )
                nc.sync.dma_start(out=yout[b * P : (b + 1) * P, :], in_=y_sb[:])

    nc.compile()
    return nc


def make_in_maps(plan, core_arrays, x, lin_w, lin_b, conf_w, conf_b, edge_index):
    n_nodes = plan["n_nodes"]
    NBP, BPC, NPAD = plan["NBP"], plan["BPC"], plan["NPAD"]
    row = np.asarray(edge_index[0], dtype=np.int64)

    x16 = np.zeros((NPAD, x.shape[1]), dtype=np.float16)
    x16[: x.shape[0]] = np.asarray(x, np.float32).astype(np.float16)
    deg = np.bincount(row, minlength=NPAD).astype(np.float32)
    degt = np.ascontiguousarray(deg.reshape(NBP, P).T)  # [128, NBP]
    CH_TOT = plan["CH_TOT"]
    conf_row = np.concatenate(
        [np.asarray(conf_w, np.float32).reshape(-1), np.asarray(conf_b, np.float32).reshape(-1)]
    ).astype(np.float32)
    confr = np.tile(conf_row, (P, 1))
    wt = np.ascontiguousarray(np.asarray(lin_w, np.float32).T.astype(np.float16))
    biasr = np.tile(np.asarray(lin_b, np.float32).reshape(1, -1), (P, 1)).astype(np.float32)
    iota = np.tile(np.arange(P, dtype=np.float16), (P, 1))
    ident = np.eye(P, dtype=np.float16)

    in_maps = []
    for c in range(plan["n_cores"]):
        arr = core_arrays[c]
        degl = np.zeros((P, CH_TOT), dtype=np.float32)
        degl[arr["lanes"] % P, arr["lanes"] // P] = deg[arr["rows"]]
        in_maps.append({
            "x16": x16,
            "degl": degl,
            "dego": np.ascontiguousarray(degt[:, c * BPC : (c + 1) * BPC]),
            "idx": arr["idx"],
            "cloc": arr["cloc"],
            "attr": arr["attr"],
            "confr": confr,
            "wt": wt,
            "biasr": biasr,
            "iota": iota,
            "ident": ident,
        })
    return in_maps


def _run(x, edge_index, edge_attr, lin_w, lin_b, conf_w, conf_b, **run_kwargs):
    n_nodes, d_in = x.shape
    d_out = lin_w.shape[0]
    plan = make_plan(edge_index, n_nodes)
    core_arrays = make_core_arrays(plan, edge_index, edge_attr)
    nc = build_program(plan, d_in, d_out)
    in_maps = make_in_maps(plan, core_arrays, x, lin_w, lin_b, conf_w, conf_b, edge_index)
    res = run_bass_kernel_spmd(nc, in_maps, list(range(plan["n_cores"])), **run_kwargs)
    ys = [np.asarray(res.results[c]["y"], np.float32) for c in range(plan["n_cores"])]
    y = np.concatenate(ys, axis=0)[:n_nodes]
    return y, res


def kernel(x, edge_index, edge_attr, lin_w, lin_b, conf_w, conf_b):
    y, _ = _run(x, edge_index, edge_attr, lin_w, lin_b, conf_w, conf_b)
    return y.astype(np.float32)


# revision 35
# speedup vs baseline: 4.5946x; 1.1006x over previous
"""Trainium2 Bass kernel for ConfidenceGCNConv message passing.

Math (reference):
    w_e   = sigmoid(edge_attr @ conf_w.T + conf_b)            # [E]
    deg   = bincount(row); dis = where(deg>0, rsqrt(deg), 0)  # [N]
    out[c] = sum_{e: col_e=c} dis[row_e]*dis[col_e]*w_e * x[row_e]
    y     = out @ lin_w.T + lin_b

Device strategy (8 cores, destination-partitioned):
  - Host: partition edges by destination block (128 dests / block, 49 blocks
    per core), split per block into row<HALF / row>=HALF halves (int16 gather
    index limit), pad each half to whole 128-lane chunks. Pure index work.
  - Device per core, per destination block b:
      * dma_gather x16[row_e] rows (fp16, 1KB each) -> Xg [128e, C, 512]
      * dma_gather dis_wide[row_e] (256B rows, col 0 = dis[row]) -> Dr
      * Wsel[e, d] = w_e * dis[row_e] * (col_local_e == d)   (DVE iota-compare)
      * PSUM spmm[128d, 512k] += sum_c Wsel_c^T @ Xg_c       (PE, contraction=edges)
      * transpose spmm -> outT [128k, 4, 128d]               (PE identity trick)
      * PSUM y[128d, 512o] = sum_kc outT_kc^T @ WT_kc        (PE)
      * y_sb = y * dis[dest] + bias                          (scale commutes with @W)
  - w_e = sigmoid(...) and dis = masked 1/sqrt(deg) computed on device.
    dis_wide (replicated 128-wide dis table for 256B-aligned gathers) is
    built on device once per core.
"""

import sys

for _p in ("/opt/trn_rl_repo",):
    if _p not in sys.path:
        sys.path.insert(0, _p)

import numpy as np

import concourse.bass as bass
import concourse.mybir as mybir
import concourse.tile as tile
from concourse import bacc
from concourse.bass_utils import run_bass_kernel_spmd

P = 128
NCORES = 8
F16 = mybir.dt.float16
F32 = mybir.dt.float32
I16 = mybir.dt.int16


def _cdiv(a, b):
    return (a + b - 1) // b


def make_plan(edge_index, n_nodes, n_cores=NCORES):
    """Host-side integer/index preprocessing: edge partition + padded layout.

    Returns a dict with static program metadata (shared across cores) and the
    per-edge placement (core, lane) used to build per-core input arrays.
    """
    row = np.asarray(edge_index[0], dtype=np.int64)
    col = np.asarray(edge_index[1], dtype=np.int64)
    E = row.shape[0]

    NB = _cdiv(n_nodes, P)           # blocks covering all nodes
    BPC = _cdiv(NB, n_cores)         # blocks per core
    NBP = BPC * n_cores              # padded total blocks
    NPAD = NBP * P                   # padded node count (dest side)
    # int16 gather indices need both halves < 32768. Within that window,
    # bias the split so the lo side of most blocks fills exactly 8 chunks
    # (one dma_gather): fewer descriptors + fewer per-instruction overheads.
    HALF = min(n_nodes // 2, max(n_nodes - 32768, 23424))
    assert HALF <= 32768 and (n_nodes - HALF) <= 32768

    gb = col // P                    # global destination block
    core = gb // BPC
    b = gb - core * BPC              # block local to core
    half = (row >= HALF).astype(np.int64)
    key = (core * BPC + b) * 2 + half

    order = np.argsort(key, kind="stable")
    counts = np.bincount(key, minlength=2 * NBP)
    cnt3 = counts.reshape(n_cores, BPC, 2)
    chunks_needed = _cdiv(cnt3, P)
    C_LO = chunks_needed[:, :, 0].max(axis=0)  # [BPC], shared across cores
    C_HI = chunks_needed[:, :, 1].max(axis=0)
    C = C_LO + C_HI
    choff = np.zeros(BPC + 1, dtype=np.int64)
    np.cumsum(C, out=choff[1:])
    CH_TOT = int(choff[-1])
    L_TOT = CH_TOT * P

    # lane base per key (core-independent): lo at choff[b]*128, hi after lo
    lane_lo = choff[:BPC] * P
    lane_hi = lane_lo + C_LO * P
    lane_base_k = np.zeros(2 * NBP, dtype=np.int64)
    for c in range(n_cores):
        lane_base_k[(c * BPC + np.arange(BPC)) * 2 + 0] = lane_lo
        lane_base_k[(c * BPC + np.arange(BPC)) * 2 + 1] = lane_hi

    sorted_key = key[order]
    starts = np.zeros(2 * NBP, dtype=np.int64)
    np.cumsum(counts[:-1], out=starts[1:])
    pos_in_grp = np.arange(E, dtype=np.int64) - starts[sorted_key]
    lane = lane_base_k[sorted_key] + pos_in_grp
    edge_core = sorted_key // (2 * BPC)

    return dict(
        n_nodes=n_nodes, E=E, NB=NB, BPC=BPC, NBP=NBP, NPAD=NPAD, HALF=HALF,
        C_LO=C_LO.astype(int), C_HI=C_HI.astype(int), C=C.astype(int),
        choff=choff.astype(int), CH_TOT=CH_TOT, L_TOT=L_TOT,
        order=order, lane=lane, edge_core=edge_core,
        n_cores=n_cores,
    )


def make_core_arrays(plan, edge_index, edge_attr):
    """Per-core padded gather-index / col-local / edge-attr arrays."""
    row = np.asarray(edge_index[0], dtype=np.int64)
    col = np.asarray(edge_index[1], dtype=np.int64)
    HALF = plan["HALF"]
    CH_TOT = plan["CH_TOT"]
    L_TOT = plan["L_TOT"]
    out = []
    for c in range(plan["n_cores"]):
        sel = plan["edge_core"] == c
        e_sel = plan["order"][sel]
        lanes = plan["lane"][sel]
        rows = row[e_sel]
        idxval = np.where(rows < HALF, rows, rows - HALF).astype(np.int16)

        idx2d = np.zeros((16, L_TOT // 16), dtype=np.int16)
        idx2d[lanes % 16, lanes // 16] = idxval
        # HW reads the 16-partition wrap replicated across all 8 Q7 core
        # groups (sim only reads partitions 0-15; HW cores read their own).
        idx2d = np.tile(idx2d, (8, 1))

        cloc = np.full((P, CH_TOT), float(P), dtype=np.float16)  # 128 = sentinel
        cloc[lanes % P, lanes // P] = (col[e_sel] % P).astype(np.float16)

        attr3 = np.zeros((3, P, CH_TOT), dtype=np.float32)
        attr3[:, lanes % P, lanes // P] = np.asarray(edge_attr, np.float32)[e_sel].T

        out.append(dict(idx=idx2d, cloc=cloc, attr=attr3, rows=rows, lanes=lanes))
    return out


def build_program(plan, d_in, d_out, debug_mode=()):
    """Build the (SPMD-shared) Bass program.

    debug_mode: set of strings to disable pieces for HW bisection:
      'no_gather_x'  - memset xg instead of dma_gather from x16
      'no_gather_d'  - memset dr instead of dma_gather from dis_wide
      'no_diswide'   - skip dis_wide build
      'no_pe'        - skip matmul/transpose; y = bias only
    """
    debug_mode = set(debug_mode)
    import os as _os
    knob_sp = _os.environ.get("KNOB_SINGLE_PACKET", "1") == "1"
    # 4 SWDGE queues (ucode max): parallelizes Q7 descriptor generation
    # across rings, the dominant cost of the per-edge row gathers.
    knob_q = int(_os.environ.get("KNOB_QUEUES", "4"))
    n_nodes = plan["n_nodes"]
    NPAD, NBP, BPC = plan["NPAD"], plan["NBP"], plan["BPC"]
    HALF = plan["HALF"]
    C_LO, C_HI, C, choff = plan["C_LO"], plan["C_HI"], plan["C"], plan["choff"]
    CH_TOT, L_TOT = plan["CH_TOT"], plan["L_TOT"]
    CMAX = int(max(C))
    KC = d_in // P          # feature chunks (4)
    assert d_in % P == 0 and d_out % P == 0

    nc = bacc.Bacc("TRN2", num_swdge_queues=knob_q)

    x16 = nc.declare_dram_parameter("x16", [NPAD, d_in], F16, isOutput=False)
    dego = nc.declare_dram_parameter("dego", [P, BPC], F32, isOutput=False)
    degl = nc.declare_dram_parameter("degl", [P, CH_TOT], F32, isOutput=False)
    idxp = nc.declare_dram_parameter("idx", [P, L_TOT // 16], I16, isOutput=False)
    clocp = nc.declare_dram_parameter("cloc", [P, CH_TOT], F16, isOutput=False)
    attrp = nc.declare_dram_parameter("attr", [3, P, CH_TOT], F32, isOutput=False)
    confp = nc.declare_dram_parameter("confr", [P, 4], F32, isOutput=False)
    wtp = nc.declare_dram_parameter("wt", [d_in, d_out], F16, isOutput=False)
    biasp = nc.declare_dram_parameter("biasr", [P, d_out], F32, isOutput=False)
    iotap = nc.declare_dram_parameter("iota", [P, P], F16, isOutput=False)
    identp = nc.declare_dram_parameter("ident", [P, P], F16, isOutput=False)
    yout = nc.declare_dram_parameter("y", [BPC * P, d_out], F32, isOutput=True)

    with tile.TileContext(nc) as tc:
        with (
            tc.tile_pool(name="const", bufs=1) as cp,
            tc.tile_pool(name="wide", bufs=2) as wp,
            tc.tile_pool(name="gath", bufs=3) as gp,
            tc.tile_pool(name="work", bufs=3) as wk,
            tc.tile_pool(name="psum", bufs=2, space="PSUM") as pp,
        ):
            # ---- constants ----
            iota_sb = cp.tile([P, 1, P], F16)
            nc.sync.dma_start(out=iota_sb[:, 0, :], in_=iotap[:, :])
            ident_sb = cp.tile([P, P], F16)
            nc.sync.dma_start(out=ident_sb[:], in_=identp[:, :])
            conf_sb = cp.tile([P, 4], F32)
            nc.sync.dma_start(out=conf_sb[:], in_=confp[:, :])
            bias_sb = cp.tile([P, d_out], F32)
            nc.sync.dma_start(out=bias_sb[:], in_=biasp[:, :])
            wt_sb = cp.tile([P, KC, d_out], F16)
            nc.sync.dma_start(
                out=wt_sb[:], in_=wtp[:, :].rearrange("(kc p) o -> p kc o", p=P)
            )
            cloc_sb = cp.tile([P, CH_TOT], F16)
            nc.sync.dma_start(out=cloc_sb[:], in_=clocp[:, :])
            idx_sb = cp.tile([P, L_TOT // 16], I16)
            nc.sync.dma_start(out=idx_sb[:], in_=idxp[:, :])
            att_sb = cp.tile([P, 3, CH_TOT], F32)
            nc.sync.dma_start(
                out=att_sb[:], in_=attrp[:, :, :].rearrange("t p c -> p t c")
            )

            # ---- dis = where(deg>0, 1/sqrt(deg), 0) ----
            def masked_rsqrt(dst, deg_sb, n):
                safe = cp.tile(list(deg_sb.shape), F32, tag=f"rs_safe{n}")
                nc.vector.tensor_scalar_max(safe[:], deg_sb[:], 1.0)
                nc.scalar.sqrt(safe[:], safe[:])
                nc.vector.reciprocal(safe[:], safe[:])
                mask = cp.tile(list(deg_sb.shape), F32, tag=f"rs_mask{n}")
                nc.vector.tensor_scalar(
                    mask[:], deg_sb[:], 0.0, None, mybir.AluOpType.is_gt
                )
                nc.vector.tensor_tensor(
                    dst[:], safe[:], mask[:], mybir.AluOpType.mult
                )

            dego_sb = cp.tile([P, BPC], F32)
            nc.sync.dma_start(out=dego_sb[:], in_=dego[:, :])
            diso_sb = cp.tile([P, BPC], F32)
            masked_rsqrt(diso_sb, dego_sb, "o")

            # dis[row_e] per lane, from host-supplied per-lane source degree
            degl_sb = cp.tile([P, CH_TOT], F32, tag="degl")
            nc.sync.dma_start(out=degl_sb[:], in_=degl[:, :])
            disl_sb = cp.tile([P, CH_TOT], F32, tag="disl")
            masked_rsqrt(disl_sb, degl_sb, "l")

            # ---- w2 = sigmoid(attr @ conf_w + conf_b) * dis[row] per lane ----
            acc = cp.tile([P, CH_TOT], F32, tag="wacc")
            tmp = cp.tile([P, CH_TOT], F32, tag="wtmp")
            nc.vector.tensor_scalar_mul(acc[:], att_sb[:, 0, :], conf_sb[:, 0:1])
            nc.vector.tensor_scalar_mul(tmp[:], att_sb[:, 1, :], conf_sb[:, 1:2])
            nc.vector.tensor_tensor(acc[:], acc[:], tmp[:], mybir.AluOpType.add)
            nc.vector.tensor_scalar_mul(tmp[:], att_sb[:, 2, :], conf_sb[:, 2:3])
            nc.vector.tensor_tensor(acc[:], acc[:], tmp[:], mybir.AluOpType.add)
            wsig = cp.tile([P, CH_TOT], F32, tag="wsig")
            nc.scalar.activation(
                wsig[:], acc[:], mybir.ActivationFunctionType.Sigmoid,
                bias=conf_sb[:, 3:4], scale=1.0,
            )
            w_sb = cp.tile([P, CH_TOT], F16, tag="w2")
            nc.vector.tensor_tensor(
                w_sb[:], wsig[:], disl_sb[:], mybir.AluOpType.mult
            )

            # ---- per destination-block pipeline ----
            gq = [0]  # global round-robin over SWDGE queues
            for b in range(BPC):
                cb = int(C[b])
                clo, chi = int(C_LO[b]), int(C_HI[b])
                co = int(choff[b])
                y_sb = wk.tile([P, d_out], F32, tag="ysb")
                if cb == 0:
                    nc.vector.tensor_copy(out=y_sb[:], in_=bias_sb[:])
                    nc.sync.dma_start(
                        out=yout[b * P : (b + 1) * P, :], in_=y_sb[:]
                    )
                    continue

                xg = gp.tile([P, CMAX, d_in], F16, tag="xg")

                # HW dma_gather caps at 1024 indices (8 chunks) per
                # instruction (Q7 per-core batch limit) - split.
                MAXCH = 8

                def emit_gathers(dst, src_ap, dst0, idx0, nch, elem):
                    for s in range(0, nch, MAXCH):
                        n = min(MAXCH, nch - s)
                        nc.gpsimd.dma_gather(
                            dst[:, dst0 + s : dst0 + s + n, :], src_ap,
                            idx_sb[:, (idx0 + s) * 8 : (idx0 + s + n) * 8],
                            n * P, n * P, elem,
                            single_packet=knob_sp,
                            queue_num=gq[0] % knob_q,
                        )
                        gq[0] += 1

                if "no_gather_x" in debug_mode:
                    nc.vector.memset(xg[:, :cb, :], 0.25)
                else:
                    emit_gathers(xg, x16[:, :], 0, co, clo, d_in)
                    emit_gathers(xg, x16[HALF:, :], clo, co + clo, chi, d_in)

                if "no_pe" in debug_mode:
                    nc.vector.tensor_copy(out=y_sb[:], in_=bias_sb[:])
                    nc.sync.dma_start(
                        out=yout[b * P : (b + 1) * P, :], in_=y_sb[:]
                    )
                    continue

                # Wsel[e, c, d] = w_e * (col_local == d)
                wsel = wk.tile([P, CMAX, P], F16, tag="wsel")
                nc.vector.tensor_tensor(
                    wsel[:, :cb, :],
                    cloc_sb[:, co : co + cb, None].to_broadcast([P, cb, P]),
                    iota_sb[:, 0:1, :].to_broadcast([P, cb, P]),
                    mybir.AluOpType.is_equal,
                )
                nc.vector.tensor_tensor(
                    wsel[:, :cb, :],
                    wsel[:, :cb, :],
                    w_sb[:, co : co + cb, None].to_broadcast([P, cb, P]),
                    mybir.AluOpType.mult,
                )

                # SpMM: psum[d, k] += sum_e Wsel[e, d] * Xg[e, k]
                ps = pp.tile([P, d_in], F32, tag="spmm")
                for cc in range(cb):
                    nc.tensor.matmul(
                        ps[:], lhsT=wsel[:, cc, :], rhs=xg[:, cc, :],
                        start=(cc == 0), stop=(cc == cb - 1),
                    )
                spmm_sb = wk.tile([P, d_in], F16, tag="spmmsb")
                nc.scalar.copy(out=spmm_sb[:], in_=ps[:])

                # transpose [d, k] -> [k, d] per 128-chunk
                pst = pp.tile([P, KC, P], F16, tag="tr")
                for kc in range(KC):
                    nc.tensor.transpose(
                        pst[:, kc, :], spmm_sb[:, kc * P : (kc + 1) * P], ident_sb[:]
                    )
                outT = wk.tile([P, KC, P], F16, tag="outT")
                nc.scalar.copy(out=outT[:], in_=pst[:])

                # linear: y[d, o] = sum_k outT[k, d] * WT[k, o]
                py = pp.tile([P, d_out], F32, tag="ylin")
                for kc in range(KC):
                    nc.tensor.matmul(
                        py[:], lhsT=outT[:, kc, :], rhs=wt_sb[:, kc, :],
                        start=(kc == 0), stop=(kc == KC - 1),
                    )
                nc.vector.tensor_scalar_mul(y_sb[:], py[:], diso_sb[:, b : b + 1])
                nc.vector.tensor_tensor(
                    y_sb[:], y_sb[:], bias_sb[:], mybir.AluOpType.add
                )
                nc.sync.dma_start(out=yout[b * P : (b + 1) * P, :], in_=y_sb[:])

    nc.compile()
    return nc


def make_in_maps(plan, core_arrays, x, lin_w, lin_b, conf_w, conf_b, edge_index):
    n_nodes = plan["n_nodes"]
    NBP, BPC, NPAD = plan["NBP"], plan["BPC"], plan["NPAD"]
    row = np.asarray(edge_index[0], dtype=np.int64)

    x16 = np.zeros((NPAD, x.shape[1]), dtype=np.float16)
    x16[: x.shape[0]] = np.asarray(x, np.float32).astype(np.float16)
    deg = np.bincount(row, minlength=NPAD).astype(np.float32)
    degt = np.ascontiguousarray(deg.reshape(NBP, P).T)  # [128, NBP]
    CH_TOT = plan["CH_TOT"]
    conf_row = np.concatenate(
        [np.asarray(conf_w, np.float32).reshape(-1), np.asarray(conf_b, np.float32).reshape(-1)]
    ).astype(np.float32)
    confr = np.tile(conf_row, (P, 1))
    wt = np.ascontiguousarray(np.asarray(lin_w, np.float32).T.astype(np.float16))
    biasr = np.tile(np.asarray(lin_b, np.float32).reshape(1, -1), (P, 1)).astype(np.float32)
    iota = np.tile(np.arange(P, dtype=np.float16), (P, 1))
    ident = np.eye(P, dtype=np.float16)

    in_maps = []
    for c in range(plan["n_cores"]):
        arr = core_arrays[c]
        degl = np.zeros((P, CH_TOT), dtype=np.float32)
        degl[arr["lanes"] % P, arr["lanes"] // P] = deg[arr["rows"]]
        in_maps.append({
            "x16": x16,
            "degl": degl,
            "dego": np.ascontiguousarray(degt[:, c * BPC : (c + 1) * BPC]),
            "idx": arr["idx"],
            "cloc": arr["cloc"],
            "attr": arr["attr"],
            "confr": confr,
            "wt": wt,
            "biasr": biasr,
            "iota": iota,
            "ident": ident,
        })
    return in_maps


def _run(x, edge_index, edge_attr, lin_w, lin_b, conf_w, conf_b, **run_kwargs):
    n_nodes, d_in = x.shape
    d_out = lin_w.shape[0]
    plan = make_plan(edge_index, n_nodes)
    core_arrays = make_core_arrays(plan, edge_index, edge_attr)
    nc = build_program(plan, d_in, d_out)
    in_maps = make_in_maps(plan, core_arrays, x, lin_w, lin_b, conf_w, conf_b, edge_index)
    res = run_bass_kernel_spmd(nc, in_maps, list(range(plan["n_cores"])), **run_kwargs)
    ys = [np.asarray(res.results[c]["y"], np.float32) for c in range(plan["n_cores"])]
    y = np.concatenate(ys, axis=0)[:n_nodes]
    return y, res


def kernel(x, edge_index, edge_attr, lin_w, lin_b, conf_w, conf_b):
    y, _ = _run(x, edge_index, edge_attr, lin_w, lin_b, conf_w, conf_b)
    return y.astype(np.float32)


# revision 36
# speedup vs baseline: 4.7960x; 1.0438x over previous
"""Trainium2 Bass kernel for ConfidenceGCNConv message passing.

Math (reference):
    w_e   = sigmoid(edge_attr @ conf_w.T + conf_b)            # [E]
    deg   = bincount(row); dis = where(deg>0, rsqrt(deg), 0)  # [N]
    out[c] = sum_{e: col_e=c} dis[row_e]*dis[col_e]*w_e * x[row_e]
    y     = out @ lin_w.T + lin_b

Device strategy (8 cores, destination-partitioned):
  - Host: partition edges by destination block (128 dests / block, 49 blocks
    per core), split per block into row<HALF / row>=HALF halves (int16 gather
    index limit), pad each half to whole 128-lane chunks. Pure index work.
  - Device per core, per destination block b:
      * dma_gather x16[row_e] rows (fp16, 1KB each) -> Xg [128e, C, 512]
      * dma_gather dis_wide[row_e] (256B rows, col 0 = dis[row]) -> Dr
      * Wsel[e, d] = w_e * dis[row_e] * (col_local_e == d)   (DVE iota-compare)
      * PSUM spmm[128d, 512k] += sum_c Wsel_c^T @ Xg_c       (PE, contraction=edges)
      * transpose spmm -> outT [128k, 4, 128d]               (PE identity trick)
      * PSUM y[128d, 512o] = sum_kc outT_kc^T @ WT_kc        (PE)
      * y_sb = y * dis[dest] + bias                          (scale commutes with @W)
  - w_e = sigmoid(...) and dis = masked 1/sqrt(deg) computed on device.
    dis_wide (replicated 128-wide dis table for 256B-aligned gathers) is
    built on device once per core.
"""

import sys

for _p in ("/opt/trn_rl_repo",):
    if _p not in sys.path:
        sys.path.insert(0, _p)

import numpy as np

import concourse.bass as bass
import concourse.mybir as mybir
import concourse.tile as tile
from concourse import bacc
from concourse.bass_utils import run_bass_kernel_spmd

P = 128
NCORES = 8
F16 = mybir.dt.float16
F32 = mybir.dt.float32
I16 = mybir.dt.int16


def _cdiv(a, b):
    return (a + b - 1) // b


def make_plan(edge_index, n_nodes, n_cores=NCORES):
    """Host-side integer/index preprocessing: edge partition + padded layout.

    Returns a dict with static program metadata (shared across cores) and the
    per-edge placement (core, lane) used to build per-core input arrays.
    """
    row = np.asarray(edge_index[0], dtype=np.int64)
    col = np.asarray(edge_index[1], dtype=np.int64)
    E = row.shape[0]

    NB = _cdiv(n_nodes, P)           # blocks covering all nodes
    BPC = _cdiv(NB, n_cores)         # blocks per core
    NBP = BPC * n_cores              # padded total blocks
    NPAD = NBP * P                   # padded node count (dest side)
    # int16 gather indices need both halves < 32768. Within that window,
    # bias the split so the lo side of most blocks fills exactly 8 chunks
    # (one dma_gather): fewer descriptors + fewer per-instruction overheads.
    HALF = min(n_nodes // 2, max(n_nodes - 32768, 23424))
    assert HALF <= 32768 and (n_nodes - HALF) <= 32768

    gb = col // P                    # global destination block
    core = gb // BPC
    b = gb - core * BPC              # block local to core
    half = (row >= HALF).astype(np.int64)
    key = (core * BPC + b) * 2 + half

    order = np.argsort(key, kind="stable")
    counts = np.bincount(key, minlength=2 * NBP)
    cnt3 = counts.reshape(n_cores, BPC, 2)
    chunks_needed = _cdiv(cnt3, P)
    C_LO = chunks_needed[:, :, 0].max(axis=0)  # [BPC], shared across cores
    C_HI = chunks_needed[:, :, 1].max(axis=0)
    C = C_LO + C_HI
    choff = np.zeros(BPC + 1, dtype=np.int64)
    np.cumsum(C, out=choff[1:])
    CH_TOT = int(choff[-1])
    L_TOT = CH_TOT * P

    # lane base per key (core-independent): lo at choff[b]*128, hi after lo
    lane_lo = choff[:BPC] * P
    lane_hi = lane_lo + C_LO * P
    lane_base_k = np.zeros(2 * NBP, dtype=np.int64)
    for c in range(n_cores):
        lane_base_k[(c * BPC + np.arange(BPC)) * 2 + 0] = lane_lo
        lane_base_k[(c * BPC + np.arange(BPC)) * 2 + 1] = lane_hi

    sorted_key = key[order]
    starts = np.zeros(2 * NBP, dtype=np.int64)
    np.cumsum(counts[:-1], out=starts[1:])
    pos_in_grp = np.arange(E, dtype=np.int64) - starts[sorted_key]
    lane = lane_base_k[sorted_key] + pos_in_grp
    edge_core = sorted_key // (2 * BPC)

    return dict(
        n_nodes=n_nodes, E=E, NB=NB, BPC=BPC, NBP=NBP, NPAD=NPAD, HALF=HALF,
        C_LO=C_LO.astype(int), C_HI=C_HI.astype(int), C=C.astype(int),
        choff=choff.astype(int), CH_TOT=CH_TOT, L_TOT=L_TOT,
        order=order, lane=lane, edge_core=edge_core,
        n_cores=n_cores,
    )


def make_core_arrays(plan, edge_index, edge_attr):
    """Per-core padded gather-index / col-local / edge-attr arrays."""
    row = np.asarray(edge_index[0], dtype=np.int64)
    col = np.asarray(edge_index[1], dtype=np.int64)
    HALF = plan["HALF"]
    CH_TOT = plan["CH_TOT"]
    L_TOT = plan["L_TOT"]
    out = []
    for c in range(plan["n_cores"]):
        sel = plan["edge_core"] == c
        e_sel = plan["order"][sel]
        lanes = plan["lane"][sel]
        rows = row[e_sel]
        idxval = np.where(rows < HALF, rows, rows - HALF).astype(np.int16)

        idx2d = np.zeros((16, L_TOT // 16), dtype=np.int16)
        idx2d[lanes % 16, lanes // 16] = idxval
        # HW reads the 16-partition wrap replicated across all 8 Q7 core
        # groups (sim only reads partitions 0-15; HW cores read their own).
        idx2d = np.tile(idx2d, (8, 1))

        cloc = np.full((P, CH_TOT), float(P), dtype=np.float16)  # 128 = sentinel
        cloc[lanes % P, lanes // P] = (col[e_sel] % P).astype(np.float16)

        attr3 = np.zeros((3, P, CH_TOT), dtype=np.float32)
        attr3[:, lanes % P, lanes // P] = np.asarray(edge_attr, np.float32)[e_sel].T

        out.append(dict(idx=idx2d, cloc=cloc, attr=attr3, rows=rows, lanes=lanes))
    return out


def build_program(plan, d_in, d_out, debug_mode=()):
    """Build the (SPMD-shared) Bass program.

    debug_mode: set of strings to disable pieces for HW bisection:
      'no_gather_x'  - memset xg instead of dma_gather from x16
      'no_gather_d'  - memset dr instead of dma_gather from dis_wide
      'no_diswide'   - skip dis_wide build
      'no_pe'        - skip matmul/transpose; y = bias only
    """
    debug_mode = set(debug_mode)
    import os as _os
    knob_sp = _os.environ.get("KNOB_SINGLE_PACKET", "1") == "1"
    # 4 SWDGE queues (ucode max): parallelizes Q7 descriptor generation
    # across rings, the dominant cost of the per-edge row gathers.
    knob_q = int(_os.environ.get("KNOB_QUEUES", "4"))
    n_nodes = plan["n_nodes"]
    NPAD, NBP, BPC = plan["NPAD"], plan["NBP"], plan["BPC"]
    HALF = plan["HALF"]
    C_LO, C_HI, C, choff = plan["C_LO"], plan["C_HI"], plan["C"], plan["choff"]
    CH_TOT, L_TOT = plan["CH_TOT"], plan["L_TOT"]
    CMAX = int(max(C))
    KC = d_in // P          # feature chunks (4)
    assert d_in % P == 0 and d_out % P == 0

    nc = bacc.Bacc("TRN2", num_swdge_queues=knob_q)

    x16 = nc.declare_dram_parameter("x16", [NPAD, d_in], F16, isOutput=False)
    dego = nc.declare_dram_parameter("dego", [P, BPC], F32, isOutput=False)
    degl = nc.declare_dram_parameter("degl", [P, CH_TOT], F32, isOutput=False)
    idxp = nc.declare_dram_parameter("idx", [P, L_TOT // 16], I16, isOutput=False)
    clocp = nc.declare_dram_parameter("cloc", [P, CH_TOT], F16, isOutput=False)
    attrp = nc.declare_dram_parameter("attr", [3, P, CH_TOT], F32, isOutput=False)
    confp = nc.declare_dram_parameter("confr", [P, 4], F32, isOutput=False)
    wtp = nc.declare_dram_parameter("wt", [d_in, d_out], F16, isOutput=False)
    biasp = nc.declare_dram_parameter("biasr", [P, d_out], F32, isOutput=False)
    iotap = nc.declare_dram_parameter("iota", [P, P], F16, isOutput=False)
    identp = nc.declare_dram_parameter("ident", [P, P], F16, isOutput=False)
    yout = nc.declare_dram_parameter("y", [BPC * P, d_out], F32, isOutput=True)

    with tile.TileContext(nc) as tc:
        with (
            tc.tile_pool(name="const", bufs=1) as cp,
            tc.tile_pool(name="wide", bufs=2) as wp,
            tc.tile_pool(name="gath", bufs=4) as gp,
            tc.tile_pool(name="work", bufs=4) as wk,
            tc.tile_pool(name="psum", bufs=2, space="PSUM") as pp,
        ):
            # ---- constants ----
            iota_sb = cp.tile([P, 1, P], F16)
            nc.sync.dma_start(out=iota_sb[:, 0, :], in_=iotap[:, :])
            ident_sb = cp.tile([P, P], F16)
            nc.sync.dma_start(out=ident_sb[:], in_=identp[:, :])
            conf_sb = cp.tile([P, 4], F32)
            nc.sync.dma_start(out=conf_sb[:], in_=confp[:, :])
            bias_sb = cp.tile([P, d_out], F32)
            nc.sync.dma_start(out=bias_sb[:], in_=biasp[:, :])
            wt_sb = cp.tile([P, KC, d_out], F16)
            nc.sync.dma_start(
                out=wt_sb[:], in_=wtp[:, :].rearrange("(kc p) o -> p kc o", p=P)
            )
            cloc_sb = cp.tile([P, CH_TOT], F16)
            nc.sync.dma_start(out=cloc_sb[:], in_=clocp[:, :])
            idx_sb = cp.tile([P, L_TOT // 16], I16)
            nc.sync.dma_start(out=idx_sb[:], in_=idxp[:, :])
            att_sb = cp.tile([P, 3, CH_TOT], F32)
            nc.sync.dma_start(
                out=att_sb[:], in_=attrp[:, :, :].rearrange("t p c -> p t c")
            )

            # ---- dis = where(deg>0, 1/sqrt(deg), 0) ----
            def masked_rsqrt(dst, deg_sb, n):
                safe = cp.tile(list(deg_sb.shape), F32, tag=f"rs_safe{n}")
                nc.vector.tensor_scalar_max(safe[:], deg_sb[:], 1.0)
                nc.scalar.sqrt(safe[:], safe[:])
                nc.vector.reciprocal(safe[:], safe[:])
                mask = cp.tile(list(deg_sb.shape), F32, tag=f"rs_mask{n}")
                nc.vector.tensor_scalar(
                    mask[:], deg_sb[:], 0.0, None, mybir.AluOpType.is_gt
                )
                nc.vector.tensor_tensor(
                    dst[:], safe[:], mask[:], mybir.AluOpType.mult
                )

            dego_sb = cp.tile([P, BPC], F32)
            nc.sync.dma_start(out=dego_sb[:], in_=dego[:, :])
            diso_sb = cp.tile([P, BPC], F32)
            masked_rsqrt(diso_sb, dego_sb, "o")

            # dis[row_e] per lane, from host-supplied per-lane source degree
            degl_sb = cp.tile([P, CH_TOT], F32, tag="degl")
            nc.sync.dma_start(out=degl_sb[:], in_=degl[:, :])
            disl_sb = cp.tile([P, CH_TOT], F32, tag="disl")
            masked_rsqrt(disl_sb, degl_sb, "l")

            # ---- w2 = sigmoid(attr @ conf_w + conf_b) * dis[row] per lane ----
            acc = cp.tile([P, CH_TOT], F32, tag="wacc")
            tmp = cp.tile([P, CH_TOT], F32, tag="wtmp")
            nc.vector.tensor_scalar_mul(acc[:], att_sb[:, 0, :], conf_sb[:, 0:1])
            nc.vector.tensor_scalar_mul(tmp[:], att_sb[:, 1, :], conf_sb[:, 1:2])
            nc.vector.tensor_tensor(acc[:], acc[:], tmp[:], mybir.AluOpType.add)
            nc.vector.tensor_scalar_mul(tmp[:], att_sb[:, 2, :], conf_sb[:, 2:3])
            nc.vector.tensor_tensor(acc[:], acc[:], tmp[:], mybir.AluOpType.add)
            wsig = cp.tile([P, CH_TOT], F32, tag="wsig")
            nc.scalar.activation(
                wsig[:], acc[:], mybir.ActivationFunctionType.Sigmoid,
                bias=conf_sb[:, 3:4], scale=1.0,
            )
            w_sb = cp.tile([P, CH_TOT], F16, tag="w2")
            nc.vector.tensor_tensor(
                w_sb[:], wsig[:], disl_sb[:], mybir.AluOpType.mult
            )

            # ---- per destination-block pipeline ----
            gq = [0]  # global round-robin over SWDGE queues
            for b in range(BPC):
                cb = int(C[b])
                clo, chi = int(C_LO[b]), int(C_HI[b])
                co = int(choff[b])
                y_sb = wk.tile([P, d_out], F32, tag="ysb")
                if cb == 0:
                    nc.vector.tensor_copy(out=y_sb[:], in_=bias_sb[:])
                    nc.sync.dma_start(
                        out=yout[b * P : (b + 1) * P, :], in_=y_sb[:]
                    )
                    continue

                xg = gp.tile([P, CMAX, d_in], F16, tag="xg")

                # HW dma_gather caps at 1024 indices (8 chunks) per
                # instruction (Q7 per-core batch limit) - split.
                MAXCH = 8

                def emit_gathers(dst, src_ap, dst0, idx0, nch, elem):
                    for s in range(0, nch, MAXCH):
                        n = min(MAXCH, nch - s)
                        nc.gpsimd.dma_gather(
                            dst[:, dst0 + s : dst0 + s + n, :], src_ap,
                            idx_sb[:, (idx0 + s) * 8 : (idx0 + s + n) * 8],
                            n * P, n * P, elem,
                            single_packet=knob_sp,
                            queue_num=gq[0] % knob_q,
                        )
                        gq[0] += 1

                if "no_gather_x" in debug_mode:
                    nc.vector.memset(xg[:, :cb, :], 0.25)
                else:
                    emit_gathers(xg, x16[:, :], 0, co, clo, d_in)
                    emit_gathers(xg, x16[HALF:, :], clo, co + clo, chi, d_in)

                if "no_pe" in debug_mode:
                    nc.vector.tensor_copy(out=y_sb[:], in_=bias_sb[:])
                    nc.sync.dma_start(
                        out=yout[b * P : (b + 1) * P, :], in_=y_sb[:]
                    )
                    continue

                # Wsel[e, c, d] = w_e * (col_local == d)
                wsel = wk.tile([P, CMAX, P], F16, tag="wsel")
                nc.vector.tensor_tensor(
                    wsel[:, :cb, :],
                    cloc_sb[:, co : co + cb, None].to_broadcast([P, cb, P]),
                    iota_sb[:, 0:1, :].to_broadcast([P, cb, P]),
                    mybir.AluOpType.is_equal,
                )
                nc.vector.tensor_tensor(
                    wsel[:, :cb, :],
                    wsel[:, :cb, :],
                    w_sb[:, co : co + cb, None].to_broadcast([P, cb, P]),
                    mybir.AluOpType.mult,
                )

                # SpMM: psum[d, k] += sum_e Wsel[e, d] * Xg[e, k]
                ps = pp.tile([P, d_in], F32, tag="spmm")
                for cc in range(cb):
                    nc.tensor.matmul(
                        ps[:], lhsT=wsel[:, cc, :], rhs=xg[:, cc, :],
                        start=(cc == 0), stop=(cc == cb - 1),
                    )
                spmm_sb = wk.tile([P, d_in], F16, tag="spmmsb")
                nc.scalar.copy(out=spmm_sb[:], in_=ps[:])

                # transpose [d, k] -> [k, d] per 128-chunk
                pst = pp.tile([P, KC, P], F16, tag="tr")
                for kc in range(KC):
                    nc.tensor.transpose(
                        pst[:, kc, :], spmm_sb[:, kc * P : (kc + 1) * P], ident_sb[:]
                    )
                outT = wk.tile([P, KC, P], F16, tag="outT")
                nc.scalar.copy(out=outT[:], in_=pst[:])

                # linear: y[d, o] = sum_k outT[k, d] * WT[k, o]
                py = pp.tile([P, d_out], F32, tag="ylin")
                for kc in range(KC):
                    nc.tensor.matmul(
                        py[:], lhsT=outT[:, kc, :], rhs=wt_sb[:, kc, :],
                        start=(kc == 0), stop=(kc == KC - 1),
                    )
                nc.vector.tensor_scalar_mul(y_sb[:], py[:], diso_sb[:, b : b + 1])
                nc.vector.tensor_tensor(
                    y_sb[:], y_sb[:], bias_sb[:], mybir.AluOpType.add
                )
                nc.sync.dma_start(out=yout[b * P : (b + 1) * P, :], in_=y_sb[:])

    nc.compile()
    return nc


def make_in_maps(plan, core_arrays, x, lin_w, lin_b, conf_w, conf_b, edge_index):
    n_nodes = plan["n_nodes"]
    NBP, BPC, NPAD = plan["NBP"], plan["BPC"], plan["NPAD"]
    row = np.asarray(edge_index[0], dtype=np.int64)

    x16 = np.zeros((NPAD, x.shape[1]), dtype=np.float16)
    x16[: x.shape[0]] = np.asarray(x, np.float32).astype(np.float16)
    deg = np.bincount(row, minlength=NPAD).astype(np.float32)
    degt = np.ascontiguousarray(deg.reshape(NBP, P).T)  # [128, NBP]
    CH_TOT = plan["CH_TOT"]
    conf_row = np.concatenate(
        [np.asarray(conf_w, np.float32).reshape(-1), np.asarray(conf_b, np.float32).reshape(-1)]
    ).astype(np.float32)
    confr = np.tile(conf_row, (P, 1))
    wt = np.ascontiguousarray(np.asarray(lin_w, np.float32).T.astype(np.float16))
    biasr = np.tile(np.asarray(lin_b, np.float32).reshape(1, -1), (P, 1)).astype(np.float32)
    iota = np.tile(np.arange(P, dtype=np.float16), (P, 1))
    ident = np.eye(P, dtype=np.float16)

    in_maps = []
    for c in range(plan["n_cores"]):
        arr = core_arrays[c]
        degl = np.zeros((P, CH_TOT), dtype=np.float32)
        degl[arr["lanes"] % P, arr["lanes"] // P] = deg[arr["rows"]]
        in_maps.append({
            "x16": x16,
            "degl": degl,
            "dego": np.ascontiguousarray(degt[:, c * BPC : (c + 1) * BPC]),
            "idx": arr["idx"],
            "cloc": arr["cloc"],
            "attr": arr["attr"],
            "confr": confr,
            "wt": wt,
            "biasr": biasr,
            "iota": iota,
            "ident": ident,
        })
    return in_maps


def _run(x, edge_index, edge_attr, lin_w, lin_b, conf_w, conf_b, **run_kwargs):
    n_nodes, d_in = x.shape
    d_out = lin_w.shape[0]
    plan = make_plan(edge_index, n_nodes)
    core_arrays = make_core_arrays(plan, edge_index, edge_attr)
    nc = build_program(plan, d_in, d_out)
    in_maps = make_in_maps(plan, core_arrays, x, lin_w, lin_b, conf_w, conf_b, edge_index)
    res = run_bass_kernel_spmd(nc, in_maps, list(range(plan["n_cores"])), **run_kwargs)
    ys = [np.asarray(res.results[c]["y"], np.float32) for c in range(plan["n_cores"])]
    y = np.concatenate(ys, axis=0)[:n_nodes]
    return y, res


def kernel(x, edge_index, edge_attr, lin_w, lin_b, conf_w, conf_b):
    y, _ = _run(x, edge_index, edge_attr, lin_w, lin_b, conf_w, conf_b)
    return y.astype(np.float32)
